# revision 28
# baseline (speedup 1.0000x reference)
"""FEDformer-style DecoderLayer on 8 trn2 NeuronCores (Bass/Tile).

Sharding: data-parallel over batch (B=16 -> 2/core). FourierBlock mode
weights [D,D,64] are mode-sharded 8 ways. Each core DFTs its OWN 2 batches
(full 64 modes), an AllToAll redistributes spectra so core c holds all 16
batches for its 8 modes, mode matmuls run there, and a second AllToAll
returns per-mode spectra to the batch owner for the inverse DFT. All FFTs
are truncated DFT matmuls.

v2: all DMA'd tensors pre-laid-out host-side so every DMA is contiguous
per partition; dummy collective issued first to absorb collective
setup/launch skew; collective outputs in Shared DRAM; om gather is 4
large DMAs; circ-conv runs weights-stationary (shared across batches)
with [co, l]-major output.
"""
import sys
sys.path.insert(0, '/opt/trn_rl_repo')
import numpy as np
import ml_dtypes

import concourse.bass as bass
import concourse.bacc as bacc
import concourse.mybir as mybir
import concourse.tile as tile
from concourse.bass_utils import run_bass_kernel_spmd
from concourse.masks import make_identity

P = 128
B, L, S, D, H, M, DFF, CO = 16, 512, 1024, 512, 8, 64, 2048, 512
NC = 8
BL = B // NC            # 2 batches/core
MJ = M // NC            # 8 modes/core
DK = D // H             # 64
DT = D // P             # 4
LT = L // P             # 4
ST = S // P             # 8
FT = DFF // P           # 16
COT = CO // P           # 4

F32 = mybir.dt.float32
F32R = mybir.dt.float32r
F16 = mybir.dt.float16
F8 = mybir.dt.float8e4
BF16 = mybir.dt.bfloat16
AF = mybir.ActivationFunctionType
OP = mybir.AluOpType
AX = mybir.AxisListType

_CACHE = {}


def _build():
    nc = bacc.Bacc("TRN2", target_bir_lowering=False, debug=False, num_devices=NC)

    def din(name, shape, dt=F32):
        return nc.dram_tensor(name, shape, dt, kind="ExternalInput")

    # all host tensors pre-laid-out partition-major -> contiguous DMAs
    xtok = din("xtok", [BL, P, LT, D], BF16)     # own x token-major
    xfm = din("xfm", [P, BL, DT, L])             # own x feature-major
    crs = din("crs", [BL, P, ST, D], F16)       # cross token-major, p-major
    fwLb = din("fwLb", [P, LT, 2 * M], BF16)     # fwd DFT basis (L)
    fwLh = din("fwLh", [P, LT, 2 * M], F16)      # same basis, f16 (Q path)
    fw1024r = din("fw1024r", [P, ST, P], F16)   # fwd DFT basis (S)
    iv512r = din("iv512r", [P, L], BF16)         # inverse DFT, A2A row order
    febwr = din("febwr", [P, MJ, DT, D], F8)
    febwi = din("febwi", [P, MJ, DT, D], F8)
    wqT = din("wqT", [P, DT, D], F16)
    wkT = din("wkT", [P, DT, D], F16)
    wvT = din("wvT", [P, DT, D], F16)
    woT = din("woT", [P, DT, D], BF16)
    dcb_kq = din("dcb_kq", [P, 3, DT, 1])        # S*bk | L*bq | S*bv cols
    bo_pp = din("bo_pp", [P, DT, 1])
    wff1T = din("wff1T", [P, FT, DT, P], BF16)   # [p=d][ft][dc][ff-col]
    wff2T = din("wff2T", [P, FT, D], BF16)       # [p=ff][fc][e]
    wccT = din("wccT", [3, P, 3, DT, CO], F16)   # [trend][p=d][shift][dc][co]
    gw1T = din("gw1T", [P, 3, DT, D // 2], BF16)
    gb1 = din("gb1", [P, 3, 2, 1])
    gw2T = din("gw2T", [P, 3, 2, 4], F32R)       # col 3 zero-pad
    grow = din("grow", [1, 16])                  # gb2 x3 (4 each) | kinv(4)
    sign_r = din("sign_r", [P, 1], BF16)

    xout = nc.dram_tensor("xout", [BL, DT, P, L], F32, kind="ExternalOutput")
    # [trend][b][co-chunk][p=co][l], f16
    rtout = nc.dram_tensor("rtout", [3, BL, COT, P, L], F16,
                           kind="ExternalOutput")

    # spectra A2A: core n sends (own 2 batches, modes of dest, re/im, all d)
    cc_sp_in = nc.dram_tensor("cc_sp_in", [NC, 2, P, BL, DT, MJ], BF16)
    cc_sp_out = nc.dram_tensor("cc_sp_out", [NC, 2, P, BL, DT, MJ], BF16)
    # om A2A: core j-owner sends per-mode products back to batch owners
    cc_om_in = nc.dram_tensor("cc_om_in", [NC, 2, BL, 2, MJ, D], BF16)
    cc_om_out = nc.dram_tensor("cc_om_out", [NC, 2, BL, 2, MJ, D], BF16)

    ctxs = []

    with tile.TileContext(nc) as tc:
        def pool(name, bufs, space="SBUF"):
            cm = tc.tile_pool(name=name, bufs=bufs, space=space)
            p = cm.__enter__()
            ctxs.append(cm)
            return p

        cp = pool("cp", 1)
        act = pool("act", 1)
        fbw = pool("fbw", 2)         # ring: FEB fp8 weights -> FFN weights
        ccw = pool("ccw", 1)         # circ-conv weights (per trend)
        xsp = pool("xsp", 2)         # rotating 2MB x-stage slots
        wr2 = pool("wr2", 2)         # rotating 1MB slots: wk,wv,wq,wo
        wk1 = pool("wk1", 1)         # single-buffered transients
        wk2 = pool("wk2", 2)         # double-buffered streams
        psA = pool("psA", 4, "PSUM")
        psC = pool("psC", 2, "PSUM")
        psB = pool("psB", 2, "PSUM")

        # ---------------- constants ----------------
        ident = cp.tile([P, P], F32, tag="ident")
        make_identity(nc, ident[:])
        warmid = psB.tile([P, P], F32, tag="psB")
        nc.tensor.transpose(warmid[:], ident[:], ident[:])

        fwLb_s = cp.tile([P, LT, 2 * M], BF16, tag="fwLb")
        nc.sync.dma_start(fwLb_s[:], fwLb[:])
        fwLh_s = cp.tile([P, LT, 2 * M], F16, tag="fwLh")
        nc.sync.dma_start(fwLh_s[:], fwLh[:])
        fw1024_s = cp.tile([P, ST, P], F16, tag="fw1024")
        nc.sync.dma_start(fw1024_s[:], fw1024r[:])
        iv512_s = cp.tile([P, L], BF16, tag="iv512")
        nc.sync.dma_start(iv512_s[:], iv512r[:])
        sign_s = cp.tile([P, 1], BF16, tag="sign")
        nc.sync.dma_start(sign_s[:], sign_r[:])
        bo_s = cp.tile([P, DT, 1], F32, tag="bo")
        nc.sync.dma_start(bo_s[:], bo_pp[:])
        gb1_s = cp.tile([P, 3, 2, 1], F32, tag="gb1")
        nc.sync.dma_start(gb1_s[:], gb1[:])
        gw2_s = cp.tile([P, 3, 2, 4], F32R, tag="gw2")
        nc.sync.dma_start(gw2_s[:], gw2T[:])
        dckq_s = cp.tile([P, 3, DT, 1], F32, tag="dckq")
        nc.sync.dma_start(dckq_s[:], dcb_kq[:])
        grow_s = cp.tile([1, 16], F32, tag="grow")
        nc.sync.dma_start(grow_s[:], grow[:])
        gbc = cp.tile([P, 16], F32, tag="gbc")
        nc.gpsimd.partition_broadcast(gbc[:], grow_s[:])
        kinv_b = gbc[:, 12:15]

        # FEB weights prefetch (contiguous 2MB each)
        fwr8 = fbw.tile([P, MJ, DT, D], F8, tag="big")
        nc.sync.dma_start(fwr8[:], febwr[:])
        fwi8 = fbw.tile([P, MJ, DT, D], F8, tag="big")
        nc.sync.dma_start(fwi8[:], febwi[:])

        # gating weights resident
        gw1_s = cp.tile([P, 3, DT, D // 2], BF16, tag="gw1s")
        nc.sync.dma_start(gw1_s[:], gw1T[:])

        # FFN weights prefetch (2MB each)
        wff1_s = fbw.tile([P, FT, DT, P], BF16, tag="big")
        nc.sync.dma_start(wff1_s[:], wff1T[:])
        wff2_s = fbw.tile([P, FT, D], BF16, tag="big")
        nc.sync.dma_start(wff2_s[:], wff2T[:])

        # ============ S1: DFT own batches (full 64 modes) ===================
        qftT = wk1.tile([P, 2, NC, BL, DT, MJ], BF16, tag="om")
        for b in range(BL):
            xb = wk2.tile([P, LT, D], BF16, tag="xtok_b")
            nc.scalar.dma_start(xb[:], xtok[b])
            for dc in range(DT):
                pd = psB.tile([P, P], F32, tag="psB")
                for lc in range(LT):
                    nc.tensor.matmul(pd[:, 0:2 * M],
                                     xb[:, lc, dc * P:(dc + 1) * P],
                                     fwLb_s[:, lc, :],
                                     start=(lc == 0), stop=(lc == LT - 1))
                nc.vector.tensor_copy(
                    qftT[:, :, :, b, dc, :],
                    pd[:, 0:2 * M].rearrange("p (r n j) -> p r n j",
                                             r=2, n=NC, j=MJ))

        # C1 cross loads first (latency-critical for PE overlap), then
        # scatter, then xfm.
        def load_cx(b):
            cxb = wk1.tile([P, ST, D], F16, tag="crs_c")
            nc.scalar.dma_start(cxb[:], crs[b])
            return cxb
        cxs = {0: load_cx(0)}
        # scatter spectra to mode owners
        for n in range(NC):
            for ri in range(2):
                nc.scalar.dma_start(cc_sp_in[n, ri], qftT[:, ri, n])
        nc.gpsimd.collective_compute(
            "AllToAll", OP.bypass, replica_groups=[list(range(NC))],
            ins=[cc_sp_in[:]], outs=[cc_sp_out[:]])

        xfm_s = xsp.tile([P, BL, DT, L], F32, tag="xs")
        nc.scalar.dma_start(xfm_s[:], xfm[:])

        # ============ C1: cross DFT (overlaps spectra A2A) ==================
        crossFd = act.tile([P, DT, BL, P], F16, tag="crossFd")
        for b in range(BL):
            cxb = cxs.pop(b) if b in cxs else load_cx(b)
            pm = psA.tile([P, 512], F32, tag="psA")
            for sc in range(ST):
                nc.tensor.matmul(pm[:], fw1024_s[:, sc, :], cxb[:, sc, :],
                                 start=(sc == 0), stop=(sc == ST - 1))
            cF = wk1.tile([P, 512], F32, tag="cF")
            nc.vector.tensor_copy(cF[:], pm[:])
            for dc in range(DT):
                pt = psB.tile([P, P], F32, tag="psB")
                nc.tensor.transpose(pt[:], cF[:, dc * P:(dc + 1) * P],
                                    ident[:])
                nc.vector.tensor_copy(crossFd[:, dc, b, :], pt[:])

        # ============ K/V proj in mode space ================================
        wk_s = wr2.tile([P, DT, D], F16, tag="wr2")
        nc.scalar.dma_start(wk_s[:], wkT[:])
        wv_s = wr2.tile([P, DT, D], F16, tag="wr2")
        nc.scalar.dma_start(wv_s[:], wvT[:])

        vf_re = act.tile([DK, BL, D], BF16, tag="vf_re")
        vf_im = act.tile([DK, BL, D], BF16, tag="vf_im")
        kf_d = act.tile([P, BL, DT, P], F16, tag="kf_d")
        qf_d = act.tile([P, BL, DT, P], F16, tag="qf_d")

        for wmat, kq, dest in ((wk_s, 0, kf_d), (wv_s, 2, None)):
            for et in range(DT):
                pk = psC.tile([P, 2 * P], F32, tag="psC")
                for dc in range(DT):
                    nc.tensor.matmul(
                        pk[:], wmat[:, dc, et * P:(et + 1) * P],
                        crossFd[:, dc].rearrange("p b m -> p (b m)"),
                        start=(dc == 0), stop=(dc == DT - 1))
                if dest is not None:
                    for b in range(BL):
                        tgt = dest[:, b, et, :]
                        nc.scalar.copy(tgt, pk[:, b * P:(b + 1) * P])
                        nc.vector.tensor_add(tgt[:, 0:1], tgt[:, 0:1],
                                             dckq_s[:, kq, et, :])
                else:
                    vtmp = wk1.tile([P, BL, P], F32, tag="vtmp")
                    nc.scalar.copy(vtmp[:], pk[:])
                    for b in range(BL):
                        nc.vector.tensor_add(vtmp[:, b, 0:1], vtmp[:, b, 0:1],
                                             dckq_s[:, kq, et, :])
                        ptr = psB.tile([DK, P], F32, tag="psB")
                        nc.tensor.transpose(ptr[:], vtmp[:, b, 0:DK],
                                            ident[:])
                        nc.vector.tensor_copy(
                            vf_re[:, b, et * P:(et + 1) * P], ptr[:])
                        pti = psB.tile([DK, P], F32, tag="psB")
                        nc.tensor.transpose(pti[:], vtmp[:, b, DK:P],
                                            ident[:])
                        nc.vector.tensor_copy(
                            vf_im[:, b, et * P:(et + 1) * P], pti[:])

        # ============ A2: per-mode matmuls on gathered spectra ==============
        qA = wk1.tile([P, 2 * BL * NC, DT, MJ], BF16, tag="om")
        for n in range(NC):
            eng = nc.sync if n % 2 == 0 else nc.scalar
            eng.dma_start(
                qA[:, n * 4:n * 4 + 4],
                cc_sp_out[n].rearrange("r p b c j -> p r b c j"))
        sgall = wk1.tile([32, 2, MJ, D], BF16, tag="stg")
        for j in range(MJ):
            g1 = psA.tile([32, 512], F32, tag="psA")
            g2 = psA.tile([32, 512], F32, tag="psA")
            for dc in range(DT):
                lh = qA[:, :, dc, j]
                nc.tensor.matmul(g1[:], lh, fwr8[:, j, dc, :],
                                 start=(dc == 0), stop=(dc == DT - 1))
                nc.tensor.matmul(g2[:], lh, fwi8[:, j, dc, :],
                                 start=(dc == 0), stop=(dc == DT - 1))
            nc.vector.tensor_copy(sgall[:, 0, j, :], g1[:])
            nc.vector.tensor_copy(sgall[:, 1, j, :], g2[:])
        nc.sync.dma_start(cc_om_in[:, :, :, 0], sgall[:, 0])
        nc.sync.dma_start(cc_om_in[:, :, :, 1], sgall[:, 1])

        nc.gpsimd.collective_compute(
            "AllToAll", OP.bypass, replica_groups=[list(range(NC))],
            ins=[cc_om_in[:]], outs=[cc_om_out[:]])

        # ============ A4: om assembly, IDFT, FEB residual ===================
        # om rows ordered (a, n, j): a=0 -> "t1-like" combination rows,
        # a=1 -> "t2-like"; host iv512 perm matches.
        x0 = xsp.tile([P, BL, DT, L], F32, tag="xs")
        for b in range(BL):
            t1 = wk1.tile([P, D], BF16, tag="a2a")
            t2 = wk1.tile([P, D], BF16, tag="a2b")
            # t1 rows: 0-63 = re*re (n,j), 64-127 = im*re (n,j)
            nc.sync.dma_start(t1[0:64], cc_om_out[:, 0, b, 0])
            nc.sync.dma_start(t1[64:128], cc_om_out[:, 1, b, 0])
            # t2 rows (half-swapped): 0-63 = im*im, 64-127 = re*im
            nc.sync.dma_start(t2[0:64], cc_om_out[:, 1, b, 1])
            nc.sync.dma_start(t2[64:128], cc_om_out[:, 0, b, 1])
            om_t = wk1.tile([P, BL, D], BF16, tag="om", name="om_t")
            om = om_t[:, 0, :]
            # om[0:64] = t1 - t2 (re part), om[64:128] = t1 + t2 (im part)
            nc.vector.scalar_tensor_tensor(om[:], t2[:], sign_s[:], t1[:],
                                           op0=OP.mult, op1=OP.add)
            for et in range(DT):
                pi = psA.tile([P, 512], F32, tag="psA")
                nc.tensor.matmul(pi[:], om[:, et * P:(et + 1) * P], iv512_s[:],
                                 start=True, stop=True)
                nc.vector.tensor_add(x0[:, b, et, :], xfm_s[:, b, et, :],
                                     pi[:])

        # ============ shared decomposition block ============================
        def decomp(xin, xout_t, widx, after_b=None):
            gb2_b = gbc[:, widx * 4:widx * 4 + 3]
            wcc3 = ccw.tile([P, 3, DT, CO], F16, tag="wcc3")
            nc.sync.dma_start(wcc3[:], wccT[widx])
            gbts = []
            for b in range(BL):
                xbf = wk2.tile([P, DT, L], BF16, tag="xbf")
                nc.scalar.copy(xbf[:], xin[:, b])
                h = wk1.tile([P, 2, L], F32R, tag=f"g_h{b}")
                for ht in range(2):
                    ph = psC.tile([P, 512], F32, tag="psC")
                    for dc in range(DT):
                        nc.tensor.matmul(ph[:],
                                         gw1_s[:, widx, dc,
                                               ht * P:(ht + 1) * P],
                                         xbf[:, dc, :],
                                         start=(dc == 0), stop=(dc == DT - 1))
                    nc.scalar.activation(h[:, ht, :], ph[:], AF.Relu,
                                         bias=gb1_s[:, widx, ht, :], scale=1.0)
                pg = psB.tile([P, LT, 4], F32, tag="psB")
                for lt_i in range(LT):
                    for hc in range(2):
                        nc.tensor.matmul(pg[:, lt_i, :],
                                         h[:, hc, lt_i * P:(lt_i + 1) * P],
                                         gw2_s[:, widx, hc, :],
                                         start=(hc == 0), stop=(hc == 1),
                                         skip_group_check=True)
                gt4 = wk1.tile([P, LT, 4], F32, tag=f"g_t{b}")
                nc.vector.tensor_add(
                    gt4[:, :, 0:3], pg[:, :, 0:3],
                    gb2_b.unsqueeze(1).broadcast_to([P, LT, 3]))
                mx4 = wk1.tile([P, LT], F32, tag=f"g_mx{b}")
                nc.vector.tensor_reduce(mx4[:], gt4[:, :, 0:3], axis=AX.X,
                                        op=OP.max, negate=True)
                nc.vector.tensor_add(
                    gt4[:, :, 0:3], gt4[:, :, 0:3],
                    mx4[:].unsqueeze(2).broadcast_to([P, LT, 3]))
                nc.scalar.activation(gt4[:, :, 0:3], gt4[:, :, 0:3], AF.Exp)
                sm4 = wk1.tile([P, LT], F32, tag=f"g_sm{b}")
                nc.vector.tensor_reduce(sm4[:], gt4[:, :, 0:3], axis=AX.X,
                                        op=OP.add)
                rc4 = wk1.tile([P, LT], F32, tag=f"g_rc{b}")
                nc.vector.reciprocal(rc4[:], sm4[:])
                nc.vector.tensor_mul(
                    gt4[:, :, 0:3], gt4[:, :, 0:3],
                    rc4[:].unsqueeze(2).broadcast_to([P, LT, 3]))
                nc.vector.tensor_mul(
                    gt4[:, :, 0:3], gt4[:, :, 0:3],
                    kinv_b.unsqueeze(1).broadcast_to([P, LT, 3]))
                nc.vector.tensor_add(gt4[:, :, 1:2], gt4[:, :, 1:2],
                                     gt4[:, :, 2:3])
                nc.vector.tensor_add(gt4[:, :, 0:1], gt4[:, :, 0:1],
                                     gt4[:, :, 1:2])
                gt4e = wk1.tile([P, 3, LT], F32, tag=f"g_te{b}")
                nc.vector.tensor_copy(
                    gt4e[:], gt4[:, :, 0:3].rearrange("p l e -> p e l"))
                pgt = psB.tile([12, P], F32, tag="psB")
                nc.tensor.transpose(
                    pgt[:], gt4e[:].rearrange("p a b -> p (a b)"), ident[:])
                g16 = wk1.tile([12, P], F16, tag=f"g16{b}")
                nc.vector.tensor_copy(g16[:], pgt[:])
                gfm = wk1.tile([1, 3, L], F16, tag=f"stg{b}")
                for e in range(3):
                    nc.scalar.dma_start(gfm[:, e, :],
                                        g16[e * 4:(e + 1) * 4, :])
                gbt = wk1.tile([P, 3, L], F16, tag=f"g_gb{b}")
                for e in range(3):
                    nc.gpsimd.partition_broadcast(gbt[:, e, :], gfm[:, e, :])
                gbts.append(gbt)

            # trend + circ conv: dc-outer, b-inner so circ-conv weights
            # (stationary) are shared across both batches.
            # psum: b0 -> psA x4 (co chunks), b1 -> psC x2 + psB x2
            prs = {}
            for co in range(COT):
                prs[(0, co)] = psA.tile([P, 512], F32, tag="psA",
                                        name=f"pcc0{co}")
            prs[(1, 0)] = psC.tile([P, 512], F32, tag="psC", name="pcc10")
            prs[(1, 1)] = psC.tile([P, 512], F32, tag="psC", name="pcc11")
            prs[(1, 2)] = psB.tile([P, 512], F32, tag="psB", name="pcc12")
            prs[(1, 3)] = psB.tile([P, 512], F32, tag="psB", name="pcc13")
            for dt_i in range(DT):
                trends = []
                for b in range(BL):
                    gbt = gbts[b]
                    pad = wk1.tile([P, L + 6], F16, tag=f"d_pad{b}")
                    nc.gpsimd.memset(pad[:, 0:3], 0.0)
                    nc.gpsimd.memset(pad[:, L + 3:L + 6], 0.0)
                    nc.scalar.copy(pad[:, 3:L + 3], xin[:, b, dt_i, :])
                    sb = wk1.tile([P, L], F16, tag=f"d_s{b}")
                    tmp = wk1.tile([P, L], F16, tag=f"d_tmp{b}")
                    trend_b = wk2.tile([P, L + 2], F16, tag=f"trend{b}")
                    acc = trend_b[:, 1:L + 1]
                    nc.vector.tensor_add(sb[:], pad[:, 2:L + 2],
                                         pad[:, 3:L + 3])
                    nc.vector.tensor_add(sb[:], sb[:], pad[:, 4:L + 4])
                    nc.vector.tensor_mul(acc[:], sb[:], gbt[:, 0, :])
                    nc.vector.tensor_add(tmp[:], pad[:, 1:L + 1],
                                         pad[:, 5:L + 5])
                    nc.vector.tensor_mul(tmp[:], tmp[:], gbt[:, 1, :])
                    nc.vector.tensor_add(acc[:], acc[:], tmp[:])
                    nc.gpsimd.tensor_add(sb[:], pad[:, 0:L], pad[:, 6:L + 6])
                    nc.gpsimd.tensor_mul(sb[:], sb[:], gbt[:, 2, :])
                    nc.vector.tensor_add(acc[:], acc[:], sb[:])
                    nc.gpsimd.tensor_copy(trend_b[:, 0:1],
                                          trend_b[:, L:L + 1])
                    nc.gpsimd.tensor_copy(trend_b[:, L + 1:L + 2],
                                          trend_b[:, 1:2])
                    nc.vector.tensor_sub(xout_t[:, b, dt_i, :],
                                         xin[:, b, dt_i, :], acc[:])
                    trends.append(trend_b)
                for s in range(3):
                    for co in range(COT):
                        wslice = wcc3[:, s, dt_i, co * P:(co + 1) * P]
                        for b in range(BL):
                            nc.tensor.matmul(
                                prs[(b, co)][:],
                                wslice,
                                trends[b][:, s:s + L],
                                start=(s == 0 and dt_i == 0),
                                stop=(s == 2 and dt_i == DT - 1),
                                skip_group_check=True)
            for b in range(BL):
                for co in range(COT):
                    rst = wk1.tile([P, L], F16, tag=f"rtst{b}")
                    nc.scalar.copy(rst[:], prs[(b, co)][:])
                    nc.sync.dma_start(rtout[widx, b, co], rst[:])
            if after_b is not None:
                for b in range(BL):
                    after_b(b)

        wq_s = wr2.tile([P, DT, D], F16, tag="wr2")
        nc.scalar.dma_start(wq_s[:], wqT[:])
        x1 = xsp.tile([P, BL, DT, L], F32, tag="xs")

        def qproj_b(b):
            x1b = wk1.tile([P, DT, L], F16, tag="xq8")
            nc.scalar.copy(x1b[:], x1[:, b])
            pqf = [psA.tile([P, P], F32, tag="psA", name=f"pqf{_i}")
                   for _i in range(DT)]
            for lc in range(LT):
                pk = psC.tile([P, 512], F32, tag="psC")
                for dc in range(DT):
                    nc.tensor.matmul(pk[:],
                                     x1b[:, dc, lc * P:(lc + 1) * P],
                                     wq_s[:, dc, :],
                                     start=(dc == 0), stop=(dc == DT - 1))
                qt = wk2.tile([P, D], F16, tag="kv_tt")
                nc.scalar.copy(qt[:], pk[:])
                for dt_i in range(DT):
                    nc.tensor.matmul(pqf[dt_i][:],
                                     qt[:, dt_i * P:(dt_i + 1) * P],
                                     fwLh_s[:, lc, :],
                                     start=(lc == 0), stop=(lc == LT - 1),
                                     skip_group_check=True)
            for dt_i in range(DT):
                nc.scalar.copy(qf_d[:, b, dt_i, :], pqf[dt_i][:])
                nc.vector.tensor_add(qf_d[:, b, dt_i, 0:1],
                                     qf_d[:, b, dt_i, 0:1],
                                     dckq_s[:, 1, dt_i, :])

        decomp(x0, x1, 0, after_b=qproj_b)

        # ============ attention =============================================
        of_sb = wk1.tile([P, BL, D], BF16, tag="om")
        for b in range(BL):
            sall = wk1.tile([DK, H, M], F32, tag="s_all")
            for hh in range(H):
                blk, half = hh // 2, (hh % 2) * DK
                pS = psB.tile([DK, M], F32, tag="psB")
                for ri in range(2):
                    nc.tensor.matmul(
                        pS[:],
                        qf_d[half:half + DK, b, blk, ri * M:(ri + 1) * M],
                        kf_d[half:half + DK, b, blk, ri * M:(ri + 1) * M],
                        start=(ri == 0), stop=(ri == 1))
                nc.vector.tensor_copy(sall[:, hh, :], pS[:])
            mx = wk1.tile([DK, H], F32, tag="s_mx")
            nc.vector.tensor_reduce(mx[:], sall[:], axis=AX.X, op=OP.max,
                                    negate=True)
            nc.vector.tensor_add(
                sall[:], sall[:],
                mx[:].unsqueeze(2).broadcast_to([DK, H, M]))
            nc.scalar.activation(sall[:], sall[:], AF.Exp)
            sm = wk1.tile([DK, H], F32, tag="s_sm")
            nc.vector.tensor_reduce(sm[:], sall[:], axis=AX.X, op=OP.add)
            rc = wk1.tile([DK, H], F32, tag="s_rc")
            nc.vector.reciprocal(rc[:], sm[:])
            nc.vector.tensor_mul(
                sall[:], sall[:],
                rc[:].unsqueeze(2).broadcast_to([DK, H, M]))
            aT = wk1.tile([DK, H, M], BF16, tag="a_T")
            for hh in range(H):
                pt = psB.tile([DK, M], F32, tag="psB")
                nc.tensor.transpose(pt[:], sall[:, hh, :], ident[0:DK, 0:DK])
                nc.vector.tensor_copy(aT[:, hh, :], pt[:])
            pof = psA.tile([P, 512], F32, tag="psA")
            for hh in range(H):
                nc.tensor.matmul(pof[0:DK, hh * DK:(hh + 1) * DK],
                                 aT[:, hh, :],
                                 vf_re[:, b, hh * DK:(hh + 1) * DK],
                                 start=True, stop=True)
                nc.tensor.matmul(pof[DK:P, hh * DK:(hh + 1) * DK],
                                 aT[:, hh, :],
                                 vf_im[:, b, hh * DK:(hh + 1) * DK],
                                 start=True, stop=True)
            nc.vector.tensor_copy(of_sb[:, b, :], pof[:])

        # idft (fm) -> wo proj + bias + residual -> x2
        wo_s = wr2.tile([P, DT, D], BF16, tag="wr2")
        nc.scalar.dma_start(wo_s[:], woT[:])
        x2 = xsp.tile([P, BL, DT, L], F32, tag="xs")
        for b in range(BL):
            apre = wk1.tile([P, DT, L], BF16, tag="ap8")
            for et in range(DT):
                pi = psA.tile([P, 512], F32, tag="psA")
                nc.tensor.matmul(pi[:], of_sb[:, b, et * P:(et + 1) * P],
                                 iv512_s[:], start=True, stop=True)
                nc.scalar.activation(apre[:, et, :], pi[:], AF.Copy,
                                     scale=262144.0)
            for et in range(DT):
                po = psA.tile([P, 512], F32, tag="psA")
                for dc in range(DT):
                    nc.tensor.matmul(po[:], wo_s[:, dc, et * P:(et + 1) * P],
                                     apre[:, dc, :],
                                     start=(dc == 0), stop=(dc == DT - 1))
                nc.vector.scalar_tensor_tensor(
                    x2[:, b, et, :], po[:], bo_s[:, et, :],
                    x1[:, b, et, :], op0=OP.add, op1=OP.add)

        # ============ decomp2 / FFN / decomp3 ===============================
        x3 = xsp.tile([P, BL, DT, L], F32, tag="xs")
        x4 = xsp.tile([P, BL, DT, L], F32, tag="xs")

        def ffn_b(b):
            x3b = wk2.tile([P, DT, L], BF16, tag="xbf")
            nc.scalar.copy(x3b[:], x3[:, b])
            y_sb = wk1.tile([P, DT, D], BF16, tag="m8k")
            for f in range(4):
                h = wk1.tile([P, FT // 4, L], BF16, tag="ffn_h")
                for fi in range(FT // 4):
                    ft = f * (FT // 4) + fi
                    ph = psC.tile([P, 512], F32, tag="psC")
                    for dc in range(DT):
                        nc.tensor.matmul(ph[:], wff1_s[:, ft, dc, :],
                                         x3b[:, dc, :],
                                         start=(dc == 0), stop=(dc == DT - 1))
                    nc.scalar.activation(h[:, fi, :], ph[:], AF.Relu)
                pys = [psA.tile([P, 512], F32, tag="psA", name=f"py{_i}")
                       for _i in range(DT)]
                for fi in range(FT // 4):
                    fc = f * (FT // 4) + fi
                    for et in range(DT):
                        nc.tensor.matmul(pys[et][:],
                                         wff2_s[:, fc, et * P:(et + 1) * P],
                                         h[:, fi, :],
                                         start=(fi == 0),
                                         stop=(fi == FT // 4 - 1))
                for et in range(DT):
                    if f == 0:
                        nc.vector.tensor_copy(y_sb[:, et, :], pys[et][:])
                    else:
                        nc.vector.tensor_add(y_sb[:, et, :], y_sb[:, et, :],
                                             pys[et][:])
            for et in range(DT):
                nc.vector.tensor_add(x4[:, b, et, :],
                                     x3[:, b, et, :],
                                     y_sb[:, et, :])

        decomp(x2, x3, 1, after_b=ffn_b)
        x5 = xsp.tile([P, BL, DT, L], F32, tag="xs")

        def out_b(b):
            nc.sync.dma_start(xout[b].rearrange("c p l -> p c l"), x5[:, b])

        decomp(x4, x5, 2, after_b=out_b)

        for cm in reversed(ctxs):
            cm.__exit__(None, None, None)

    nc.compile()
    return nc


# ---------------------------------------------------------------------------
# host side
# ---------------------------------------------------------------------------
def _fwd_basis_cols(n, modes):
    l = np.arange(n)[:, None].astype(np.float64)
    m = np.asarray(modes)[None, :].astype(np.float64)
    th = 2.0 * np.pi * l * m / n
    return np.concatenate([np.cos(th), -np.sin(th)], axis=1).astype(np.float32)


def _inv_basis(n):
    l = np.arange(n)[None, :].astype(np.float64)
    m = np.arange(M)[:, None].astype(np.float64)
    c = np.where(np.arange(M) == 0, 1.0, 2.0)[:, None]
    th = 2.0 * np.pi * l * m / n
    return np.concatenate([c * np.cos(th) / n, -c * np.sin(th) / n],
                         axis=0).astype(np.float32)


def _prep_in_maps(x, cross, feb_wr, feb_wi, wq, bq, wk, bk, wv, bv, wo, bo,
                  w_ff1, w_ff2, d1_w1, d1_b1, d1_w2, d1_b2,
                  d2_w1, d2_b1, d2_w2, d2_b2, d3_w1, d3_b1, d3_w2, d3_b2,
                  p1, p2, p3):
    bf16 = ml_dtypes.bfloat16
    f8 = ml_dtypes.float8_e4m3
    x = np.ascontiguousarray(x, np.float32)
    cross = np.ascontiguousarray(cross, np.float32)

    # token-major x: [b][p=tok][lt][d]
    xtok_full = np.ascontiguousarray(
        x.reshape(B, LT, P, D).transpose(0, 2, 1, 3)).astype(bf16)
    # feature-major x: [p=d%128][b][dc][l] (per-core slice along b)
    xfm_t = np.ascontiguousarray(x.transpose(0, 2, 1)).reshape(B, DT, P, L)
    crs_full = np.ascontiguousarray(
        cross.reshape(B, ST, P, D).transpose(0, 2, 1, 3)).astype(np.float16)

    fwL_f32 = _fwd_basis_cols(L, np.arange(M))          # [L, 2M]
    fwL_pm = np.ascontiguousarray(
        fwL_f32.reshape(LT, P, 2 * M).transpose(1, 0, 2))   # [P, LT, 2M]
    fwLb_np = fwL_pm.astype(bf16)
    fwLh_np = fwL_pm.astype(np.float16)
    fw1024r_np = np.ascontiguousarray(
        _fwd_basis_cols(S, np.arange(M)).reshape(ST, P, P).transpose(1, 0, 2)) \
        .astype(np.float16)
    iv512_np = _inv_basis(L)
    # om rows arrive as (a, n, j): row a*64 + n*8 + j holds
    # (re if a==0 else im) of global mode n*8+j
    perm = np.zeros(P, np.int64)
    for a in range(2):
        for n_i in range(NC):
            for j_i in range(MJ):
                perm[a * 64 + n_i * MJ + j_i] = a * M + n_i * MJ + j_i
    iv512_np = np.ascontiguousarray(iv512_np[perm] / 262144.0).astype(bf16)

    def pm3(w):  # [D, D] -> [P, DT, D] partition-major transposed
        return np.ascontiguousarray(
            np.asarray(w).T.reshape(DT, P, D).transpose(1, 0, 2))

    wqT_np = pm3(wq).astype(np.float16)
    wkT_np = pm3(wk).astype(np.float16)
    wvT_np = pm3(wv).astype(np.float16)
    woT_np = pm3(wo).astype(bf16)
    dcb_kq_np = np.ascontiguousarray(
        np.stack([np.asarray(bk) * S, np.asarray(bq) * L,
                  np.asarray(bv) * S]).reshape(3, DT, P, 1)
        .transpose(2, 0, 1, 3)).astype(np.float32)
    bo_np = np.ascontiguousarray(
        np.asarray(bo).reshape(DT, P, 1).transpose(1, 0, 2)).astype(np.float32)
    # [p=d][ft][dc][o]
    wff1_np = np.ascontiguousarray(
        np.asarray(w_ff1).T.reshape(DT, P, FT, P).transpose(1, 2, 0, 3)) \
        .astype(bf16)
    # [p=ff][fc][e]
    wff2_np = np.ascontiguousarray(
        np.asarray(w_ff2).T.reshape(FT, P, D).transpose(1, 0, 2)).astype(bf16)
    # [w][p=d][s][dc][co]
    wcc_np = np.zeros((3, P, 3, DT, CO), np.float16)
    for w_i, p_i in enumerate((p1, p2, p3)):
        for s in range(3):
            wcc_np[w_i, :, s] = np.ascontiguousarray(p_i[:, :, s].T) \
                .reshape(DT, P, CO).transpose(1, 0, 2)
    gw1_np = np.ascontiguousarray(
        np.stack([np.asarray(w).T.reshape(DT, P, D // 2)
                  for w in (d1_w1, d2_w1, d3_w1)])
        .transpose(2, 0, 1, 3)).astype(bf16)
    gb1_np = np.ascontiguousarray(
        np.stack([np.asarray(b_).reshape(2, P, 1)
                  for b_ in (d1_b1, d2_b1, d3_b1)])
        .transpose(2, 0, 1, 3)).astype(np.float32)
    gw2_np = np.zeros((3, 2, P, 4), np.float32)
    for i, w in enumerate((d1_w2, d2_w2, d3_w2)):
        gw2_np[i, :, :, 0:3] = np.ascontiguousarray(np.asarray(w).T) \
            .reshape(2, P, 3)
    gw2_np = np.ascontiguousarray(gw2_np.transpose(2, 0, 1, 3))
    grow_np = np.zeros((1, 16), np.float32)
    for i, b2 in enumerate((d1_b2, d2_b2, d3_b2)):
        grow_np[0, i * 4:i * 4 + 3] = np.asarray(b2, np.float32)
    grow_np[0, 12:15] = [1.0 / 3.0, 1.0 / 5.0, 1.0 / 7.0]
    # om combine: om[0:64] = t1 - t2, om[64:128] = t1 + t2
    sign_np = np.concatenate([-np.ones(64), np.ones(64)]) \
        .reshape(P, 1).astype(bf16)

    def febp(w):  # [D, D, M] -> per-core [P, MJ, DT, D] fp8
        outs = []
        for c in range(NC):
            t = np.asarray(w)[:, :, MJ * c:MJ * (c + 1)].transpose(2, 0, 1)
            t = (t * 262144.0).reshape(MJ, DT, P, D).transpose(2, 0, 1, 3)
            outs.append(np.ascontiguousarray(t).astype(f8))
        return outs

    febwr_l, febwi_l = febp(feb_wr), febp(feb_wi)

    in_maps = []
    for c in range(NC):
        bs = slice(BL * c, BL * (c + 1))
        in_maps.append(dict(
            xtok=xtok_full[bs],
            xfm=np.ascontiguousarray(xfm_t[bs].transpose(2, 0, 1, 3)),
            crs=crs_full[bs],
            fwLb=fwLb_np, fwLh=fwLh_np,
            fw1024r=fw1024r_np, iv512r=iv512_np,
            febwr=febwr_l[c], febwi=febwi_l[c],
            wqT=wqT_np, wkT=wkT_np, wvT=wvT_np, woT=woT_np,
            dcb_kq=dcb_kq_np, bo_pp=bo_np,
            wff1T=wff1_np, wff2T=wff2_np, wccT=wcc_np,
            gw1T=gw1_np, gb1=gb1_np, gw2T=gw2_np,
            grow=grow_np, sign_r=sign_np,
        ))

    return in_maps


def kernel(**inputs):
    if "nc" not in _CACHE:
        _CACHE["nc"] = _build()
    nc = _CACHE["nc"]
    in_maps = _prep_in_maps(**inputs)
    _CACHE["in_maps"] = in_maps
    res = run_bass_kernel_spmd(nc, in_maps, list(range(NC)))
    xo = np.zeros((B, L, D), np.float32)
    rt = np.zeros((B, L, CO), np.float32)
    for c in range(NC):
        r = res.results[c]
        xo[BL * c:BL * (c + 1)] = np.asarray(r["xout"]) \
            .reshape(BL, D, L).transpose(0, 2, 1)
        # rtout [3, BL, COT, P(co), L] f16 -> [BL, L, CO]
        rt[BL * c:BL * (c + 1)] = np.asarray(r["rtout"]).astype(np.float32) \
            .sum(axis=0).reshape(BL, CO, L).transpose(0, 2, 1)
    return xo, rt


# revision 29
# speedup vs baseline: 1.3518x; 1.3518x over previous
"""FEDformer-style DecoderLayer on 8 trn2 NeuronCores (Bass/Tile).

Sharding: data-parallel over batch (B=16 -> 2/core). FourierBlock mode
weights [D,D,64] are mode-sharded 8 ways. Each core DFTs its OWN 2 batches
(full 64 modes), an AllToAll redistributes spectra so core c holds all 16
batches for its 8 modes, mode matmuls run there, and a second AllToAll
returns per-mode spectra to the batch owner for the inverse DFT. All FFTs
are truncated DFT matmuls.

v2: all DMA'd tensors pre-laid-out host-side so every DMA is contiguous
per partition; dummy collective issued first to absorb collective
setup/launch skew; collective outputs in Shared DRAM; om gather is 4
large DMAs; circ-conv runs weights-stationary (shared across batches)
with [co, l]-major output.
"""
import sys
sys.path.insert(0, '/opt/trn_rl_repo')
import numpy as np
import ml_dtypes

import concourse.bass as bass
import concourse.bacc as bacc
import concourse.mybir as mybir
import concourse.tile as tile
from concourse.bass_utils import run_bass_kernel_spmd
from concourse.masks import make_identity

P = 128
B, L, S, D, H, M, DFF, CO = 16, 512, 1024, 512, 8, 64, 2048, 512
NC = 8
BL = B // NC            # 2 batches/core
MJ = M // NC            # 8 modes/core
DK = D // H             # 64
DT = D // P             # 4
LT = L // P             # 4
ST = S // P             # 8
FT = DFF // P           # 16
COT = CO // P           # 4

F32 = mybir.dt.float32
F32R = mybir.dt.float32r
F16 = mybir.dt.float16
F8 = mybir.dt.float8e4
BF16 = mybir.dt.bfloat16
AF = mybir.ActivationFunctionType
OP = mybir.AluOpType
AX = mybir.AxisListType

_CACHE = {}


def _build():
    nc = bacc.Bacc("TRN2", target_bir_lowering=False, debug=False, num_devices=NC)

    def din(name, shape, dt=F32):
        return nc.dram_tensor(name, shape, dt, kind="ExternalInput")

    # all host tensors pre-laid-out partition-major -> contiguous DMAs
    xtok = din("xtok", [BL, P, LT, D], BF16)     # own x token-major
    xfm = din("xfm", [P, BL, DT, L])             # own x feature-major
    crs = din("crs", [BL, P, ST, D], F16)       # cross token-major, p-major
    fwLb = din("fwLb", [P, LT, 2 * M], BF16)     # fwd DFT basis (L)
    fwLh = din("fwLh", [P, LT, 2 * M], F16)      # same basis, f16 (Q path)
    fw1024r = din("fw1024r", [P, ST, P], F16)   # fwd DFT basis (S)
    iv512r = din("iv512r", [P, L], BF16)         # inverse DFT, A2A row order
    febwr = din("febwr", [P, MJ, DT, D], F8)
    febwi = din("febwi", [P, MJ, DT, D], F8)
    wqT = din("wqT", [P, DT, D], F16)
    wkT = din("wkT", [P, DT, D], F16)
    wvT = din("wvT", [P, DT, D], F16)
    woT = din("woT", [P, DT, D], BF16)
    dcb_kq = din("dcb_kq", [P, 3, DT, 1])        # S*bk | L*bq | S*bv cols
    bo_pp = din("bo_pp", [P, DT, 1])
    wff1T = din("wff1T", [P, FT, DT, P], BF16)   # [p=d][ft][dc][ff-col]
    wff2T = din("wff2T", [P, FT, D], BF16)       # [p=ff][fc][e]
    wccT = din("wccT", [3, P, 3, DT, CO], F16)   # [trend][p=d][shift][dc][co]
    gw1T = din("gw1T", [P, 3, DT, D // 2], BF16)
    gb1 = din("gb1", [P, 3, 2, 1])
    gw2T = din("gw2T", [P, 3, 2, 4], F32R)       # col 3 zero-pad
    grow = din("grow", [1, 16])                  # gb2 x3 (4 each) | kinv(4)
    sign_r = din("sign_r", [P, 1], BF16)

    xout = nc.dram_tensor("xout", [BL, DT, P, L], F32, kind="ExternalOutput")
    # [trend][b][co-chunk][p=co][l], f16
    rtout = nc.dram_tensor("rtout", [3, BL, COT, P, L], F16,
                           kind="ExternalOutput")

    # spectra A2A: core n sends (own 2 batches, modes of dest, re/im, all d)
    cc_sp_in = nc.dram_tensor("cc_sp_in", [NC, 2, P, BL, DT, MJ], BF16)
    cc_sp_out = nc.dram_tensor("cc_sp_out", [NC, 2, P, BL, DT, MJ], BF16)
    # om A2A: core j-owner sends per-mode products back to batch owners
    cc_om_in = nc.dram_tensor("cc_om_in", [NC, 2, BL, 2, MJ, D], BF16)
    cc_om_out = nc.dram_tensor("cc_om_out", [NC, 2, BL, 2, MJ, D], BF16)

    ctxs = []

    with tile.TileContext(nc) as tc:
        def pool(name, bufs, space="SBUF"):
            cm = tc.tile_pool(name=name, bufs=bufs, space=space)
            p = cm.__enter__()
            ctxs.append(cm)
            return p

        cp = pool("cp", 1)
        act = pool("act", 1)
        fbw = pool("fbw", 2)         # ring: FEB fp8 weights -> FFN weights
        ccw = pool("ccw", 1)         # circ-conv weights (per trend)
        xsp = pool("xsp", 2)         # rotating 2MB x-stage slots
        wr2 = pool("wr2", 2)         # rotating 1MB slots: wk,wv,wq,wo
        wk1 = pool("wk1", 1)         # single-buffered transients
        wk2 = pool("wk2", 2)         # double-buffered streams
        psA = pool("psA", 4, "PSUM")
        psC = pool("psC", 2, "PSUM")
        psB = pool("psB", 2, "PSUM")

        # ---------------- constants ----------------
        ident = cp.tile([P, P], F32, tag="ident")
        make_identity(nc, ident[:])
        warmid = psB.tile([P, P], F32, tag="psB")
        nc.tensor.transpose(warmid[:], ident[:], ident[:])

        fwLb_s = cp.tile([P, LT, 2 * M], BF16, tag="fwLb")
        nc.sync.dma_start(fwLb_s[:], fwLb[:])
        fwLh_s = cp.tile([P, LT, 2 * M], F16, tag="fwLh")
        nc.sync.dma_start(fwLh_s[:], fwLh[:])
        fw1024_s = cp.tile([P, ST, P], F16, tag="fw1024")
        nc.sync.dma_start(fw1024_s[:], fw1024r[:])
        iv512_s = cp.tile([P, L], BF16, tag="iv512")
        nc.sync.dma_start(iv512_s[:], iv512r[:])
        sign_s = cp.tile([P, 1], BF16, tag="sign")
        nc.sync.dma_start(sign_s[:], sign_r[:])
        bo_s = cp.tile([P, DT, 1], F32, tag="bo")
        nc.sync.dma_start(bo_s[:], bo_pp[:])
        gb1_s = cp.tile([P, 3, 2, 1], F32, tag="gb1")
        nc.sync.dma_start(gb1_s[:], gb1[:])
        gw2_s = cp.tile([P, 3, 2, 4], F32R, tag="gw2")
        nc.sync.dma_start(gw2_s[:], gw2T[:])
        dckq_s = cp.tile([P, 3, DT, 1], F32, tag="dckq")
        nc.sync.dma_start(dckq_s[:], dcb_kq[:])
        grow_s = cp.tile([1, 16], F32, tag="grow")
        nc.sync.dma_start(grow_s[:], grow[:])
        gbc = cp.tile([P, 16], F32, tag="gbc")
        nc.gpsimd.partition_broadcast(gbc[:], grow_s[:])
        kinv_b = gbc[:, 12:15]

        # FEB weights prefetch (contiguous 2MB each)
        fwr8 = fbw.tile([P, MJ, DT, D], F8, tag="big")
        nc.sync.dma_start(fwr8[:], febwr[:])
        fwi8 = fbw.tile([P, MJ, DT, D], F8, tag="big")
        nc.sync.dma_start(fwi8[:], febwi[:])

        # gating weights resident
        gw1_s = cp.tile([P, 3, DT, D // 2], BF16, tag="gw1s")
        nc.sync.dma_start(gw1_s[:], gw1T[:])

        # FFN weights prefetch (2MB each)
        wff1_s = fbw.tile([P, FT, DT, P], BF16, tag="big")
        nc.sync.dma_start(wff1_s[:], wff1T[:])
        wff2_s = fbw.tile([P, FT, D], BF16, tag="big")
        nc.sync.dma_start(wff2_s[:], wff2T[:])

        # ============ S1: DFT own batches (full 64 modes) ===================
        qftT = wk1.tile([P, 2, NC, BL, DT, MJ], BF16, tag="om")
        for b in range(BL):
            xb = wk2.tile([P, LT, D], BF16, tag="xtok_b")
            nc.scalar.dma_start(xb[:], xtok[b])
            for dc in range(DT):
                pd = psB.tile([P, P], F32, tag="psB")
                for lc in range(LT):
                    nc.tensor.matmul(pd[:, 0:2 * M],
                                     xb[:, lc, dc * P:(dc + 1) * P],
                                     fwLb_s[:, lc, :],
                                     start=(lc == 0), stop=(lc == LT - 1))
                nc.vector.tensor_copy(
                    qftT[:, :, :, b, dc, :],
                    pd[:, 0:2 * M].rearrange("p (r n j) -> p r n j",
                                             r=2, n=NC, j=MJ))

        # C1 cross loads first (latency-critical for PE overlap), then
        # scatter, then xfm.
        def load_cx(b):
            cxb = wk1.tile([P, ST, D], F16, tag="crs_c")
            nc.scalar.dma_start(cxb[:], crs[b])
            return cxb
        cxs = {0: load_cx(0)}
        # scatter spectra to mode owners
        for n in range(NC):
            for ri in range(2):
                nc.scalar.dma_start(cc_sp_in[n, ri], qftT[:, ri, n])
        nc.gpsimd.collective_compute(
            "AllToAll", OP.bypass, replica_groups=[list(range(NC))],
            ins=[cc_sp_in[:]], outs=[cc_sp_out[:]])

        xfm_s = xsp.tile([P, BL, DT, L], F32, tag="xs")
        nc.scalar.dma_start(xfm_s[:], xfm[:])

        # ============ C1: cross DFT (overlaps spectra A2A) ==================
        crossFd = act.tile([P, DT, BL, P], F16, tag="crossFd")
        for b in range(BL):
            cxb = cxs.pop(b) if b in cxs else load_cx(b)
            pm = psA.tile([P, 512], F32, tag="psA")
            for sc in range(ST):
                nc.tensor.matmul(pm[:], fw1024_s[:, sc, :], cxb[:, sc, :],
                                 start=(sc == 0), stop=(sc == ST - 1))
            cF = wk1.tile([P, 512], F32, tag="cF")
            nc.vector.tensor_copy(cF[:], pm[:])
            for dc in range(DT):
                pt = psB.tile([P, P], F32, tag="psB")
                nc.tensor.transpose(pt[:], cF[:, dc * P:(dc + 1) * P],
                                    ident[:])
                nc.vector.tensor_copy(crossFd[:, dc, b, :], pt[:])

        # ============ K/V proj in mode space ================================
        wk_s = wr2.tile([P, DT, D], F16, tag="wr2")
        nc.scalar.dma_start(wk_s[:], wkT[:])
        wv_s = wr2.tile([P, DT, D], F16, tag="wr2")
        nc.scalar.dma_start(wv_s[:], wvT[:])

        vf_re = act.tile([DK, BL, D], BF16, tag="vf_re")
        vf_im = act.tile([DK, BL, D], BF16, tag="vf_im")
        kf_d = act.tile([P, BL, DT, P], F16, tag="kf_d")
        qf_d = act.tile([P, BL, DT, P], F16, tag="qf_d")

        for wmat, kq, dest in ((wk_s, 0, kf_d), (wv_s, 2, None)):
            for et in range(DT):
                pk = psC.tile([P, 2 * P], F32, tag="psC")
                for dc in range(DT):
                    nc.tensor.matmul(
                        pk[:], wmat[:, dc, et * P:(et + 1) * P],
                        crossFd[:, dc].rearrange("p b m -> p (b m)"),
                        start=(dc == 0), stop=(dc == DT - 1))
                if dest is not None:
                    for b in range(BL):
                        tgt = dest[:, b, et, :]
                        nc.scalar.copy(tgt, pk[:, b * P:(b + 1) * P])
                        nc.vector.tensor_add(tgt[:, 0:1], tgt[:, 0:1],
                                             dckq_s[:, kq, et, :])
                else:
                    vtmp = wk1.tile([P, BL, P], F32, tag="vtmp")
                    nc.scalar.copy(vtmp[:], pk[:])
                    for b in range(BL):
                        nc.vector.tensor_add(vtmp[:, b, 0:1], vtmp[:, b, 0:1],
                                             dckq_s[:, kq, et, :])
                        ptr = psB.tile([DK, P], F32, tag="psB")
                        nc.tensor.transpose(ptr[:], vtmp[:, b, 0:DK],
                                            ident[:])
                        nc.vector.tensor_copy(
                            vf_re[:, b, et * P:(et + 1) * P], ptr[:])
                        pti = psB.tile([DK, P], F32, tag="psB")
                        nc.tensor.transpose(pti[:], vtmp[:, b, DK:P],
                                            ident[:])
                        nc.vector.tensor_copy(
                            vf_im[:, b, et * P:(et + 1) * P], pti[:])

        # ============ A2: per-mode matmuls on gathered spectra ==============
        qA = wk1.tile([P, 2 * BL * NC, DT, MJ], BF16, tag="om")
        for n in range(NC):
            eng = nc.sync if n % 2 == 0 else nc.scalar
            eng.dma_start(
                qA[:, n * 4:n * 4 + 4],
                cc_sp_out[n].rearrange("r p b c j -> p r b c j"))
        sgall = wk1.tile([32, 2, MJ, D], BF16, tag="stg")
        for j in range(MJ):
            g1 = psA.tile([32, 512], F32, tag="psA")
            g2 = psA.tile([32, 512], F32, tag="psA")
            for dc in range(DT):
                lh = qA[:, :, dc, j]
                nc.tensor.matmul(g1[:], lh, fwr8[:, j, dc, :],
                                 start=(dc == 0), stop=(dc == DT - 1))
                nc.tensor.matmul(g2[:], lh, fwi8[:, j, dc, :],
                                 start=(dc == 0), stop=(dc == DT - 1))
            nc.vector.tensor_copy(sgall[:, 0, j, :], g1[:])
            nc.vector.tensor_copy(sgall[:, 1, j, :], g2[:])
        nc.sync.dma_start(cc_om_in[:, :, :, 0], sgall[:, 0])
        nc.sync.dma_start(cc_om_in[:, :, :, 1], sgall[:, 1])

        nc.gpsimd.collective_compute(
            "AllToAll", OP.bypass, replica_groups=[list(range(NC))],
            ins=[cc_om_in[:]], outs=[cc_om_out[:]])

        # ============ A4: om assembly, IDFT, FEB residual ===================
        # om rows ordered (a, n, j): a=0 -> "t1-like" combination rows,
        # a=1 -> "t2-like"; host iv512 perm matches.
        x0 = xsp.tile([P, BL, DT, L], F32, tag="xs")
        for b in range(BL):
            t1 = wk1.tile([P, D], BF16, tag="a2a")
            t2 = wk1.tile([P, D], BF16, tag="a2b")
            # t1 rows: 0-63 = re*re (n,j), 64-127 = im*re (n,j)
            nc.sync.dma_start(t1[0:64], cc_om_out[:, 0, b, 0])
            nc.sync.dma_start(t1[64:128], cc_om_out[:, 1, b, 0])
            # t2 rows (half-swapped): 0-63 = im*im, 64-127 = re*im
            nc.sync.dma_start(t2[0:64], cc_om_out[:, 1, b, 1])
            nc.sync.dma_start(t2[64:128], cc_om_out[:, 0, b, 1])
            om_t = wk1.tile([P, BL, D], BF16, tag="om", name="om_t")
            om = om_t[:, 0, :]
            # om[0:64] = t1 - t2 (re part), om[64:128] = t1 + t2 (im part)
            nc.vector.scalar_tensor_tensor(om[:], t2[:], sign_s[:], t1[:],
                                           op0=OP.mult, op1=OP.add)
            for et in range(DT):
                pi = psA.tile([P, 512], F32, tag="psA")
                nc.tensor.matmul(pi[:], om[:, et * P:(et + 1) * P], iv512_s[:],
                                 start=True, stop=True)
                nc.vector.tensor_add(x0[:, b, et, :], xfm_s[:, b, et, :],
                                     pi[:])

        # ============ shared decomposition block ============================
        def decomp(xin, xout_t, widx, after_b=None):
            gb2_b = gbc[:, widx * 4:widx * 4 + 3]
            wcc3 = ccw.tile([P, 3, DT, CO], F16, tag="wcc3")
            nc.sync.dma_start(wcc3[:], wccT[widx])
            gbts = []
            for b in range(BL):
                xbf = wk2.tile([P, DT, L], BF16, tag="xbf")
                nc.scalar.copy(xbf[:], xin[:, b])
                h = wk1.tile([P, 2, L], F32R, tag=f"g_h{b}")
                for ht in range(2):
                    ph = psC.tile([P, 512], F32, tag="psC")
                    for dc in range(DT):
                        nc.tensor.matmul(ph[:],
                                         gw1_s[:, widx, dc,
                                               ht * P:(ht + 1) * P],
                                         xbf[:, dc, :],
                                         start=(dc == 0), stop=(dc == DT - 1))
                    nc.scalar.activation(h[:, ht, :], ph[:], AF.Relu,
                                         bias=gb1_s[:, widx, ht, :], scale=1.0)
                pg = psB.tile([P, LT, 4], F32, tag="psB")
                for lt_i in range(LT):
                    for hc in range(2):
                        nc.tensor.matmul(pg[:, lt_i, :],
                                         h[:, hc, lt_i * P:(lt_i + 1) * P],
                                         gw2_s[:, widx, hc, :],
                                         start=(hc == 0), stop=(hc == 1),
                                         skip_group_check=True)
                gt4 = wk1.tile([P, LT, 4], F32, tag=f"g_t{b}")
                nc.vector.tensor_add(
                    gt4[:, :, 0:3], pg[:, :, 0:3],
                    gb2_b.unsqueeze(1).broadcast_to([P, LT, 3]))
                mx4 = wk1.tile([P, LT], F32, tag=f"g_mx{b}")
                nc.vector.tensor_reduce(mx4[:], gt4[:, :, 0:3], axis=AX.X,
                                        op=OP.max, negate=True)
                nc.vector.tensor_add(
                    gt4[:, :, 0:3], gt4[:, :, 0:3],
                    mx4[:].unsqueeze(2).broadcast_to([P, LT, 3]))
                nc.scalar.activation(gt4[:, :, 0:3], gt4[:, :, 0:3], AF.Exp)
                sm4 = wk1.tile([P, LT], F32, tag=f"g_sm{b}")
                nc.vector.tensor_reduce(sm4[:], gt4[:, :, 0:3], axis=AX.X,
                                        op=OP.add)
                rc4 = wk1.tile([P, LT], F32, tag=f"g_rc{b}")
                nc.vector.reciprocal(rc4[:], sm4[:])
                nc.vector.tensor_mul(
                    gt4[:, :, 0:3], gt4[:, :, 0:3],
                    rc4[:].unsqueeze(2).broadcast_to([P, LT, 3]))
                nc.vector.tensor_mul(
                    gt4[:, :, 0:3], gt4[:, :, 0:3],
                    kinv_b.unsqueeze(1).broadcast_to([P, LT, 3]))
                nc.vector.tensor_add(gt4[:, :, 1:2], gt4[:, :, 1:2],
                                     gt4[:, :, 2:3])
                nc.vector.tensor_add(gt4[:, :, 0:1], gt4[:, :, 0:1],
                                     gt4[:, :, 1:2])
                gt4e = wk1.tile([P, 3, LT], F32, tag=f"g_te{b}")
                nc.vector.tensor_copy(
                    gt4e[:], gt4[:, :, 0:3].rearrange("p l e -> p e l"))
                pgt = psB.tile([12, P], F32, tag="psB")
                nc.tensor.transpose(
                    pgt[:], gt4e[:].rearrange("p a b -> p (a b)"), ident[:])
                g16 = wk1.tile([12, P], F16, tag=f"g16{b}")
                nc.vector.tensor_copy(g16[:], pgt[:])
                gfm = wk1.tile([1, 3, L], F16, tag=f"stg{b}")
                for e in range(3):
                    nc.scalar.dma_start(gfm[:, e, :],
                                        g16[e * 4:(e + 1) * 4, :])
                gbt = wk1.tile([P, 3, L], F16, tag=f"g_gb{b}")
                for e in range(3):
                    nc.gpsimd.partition_broadcast(gbt[:, e, :], gfm[:, e, :])
                gbts.append(gbt)

            # trend + circ conv: dc-outer, b-inner so circ-conv weights
            # (stationary) are shared across both batches.
            # psum: b0 -> psA x4 (co chunks), b1 -> psC x2 + psB x2
            prs = {}
            for co in range(COT):
                prs[(0, co)] = psA.tile([P, 512], F32, tag="psA",
                                        name=f"pcc0{co}")
            prs[(1, 0)] = psC.tile([P, 512], F32, tag="psC", name="pcc10")
            prs[(1, 1)] = psC.tile([P, 512], F32, tag="psC", name="pcc11")
            prs[(1, 2)] = psB.tile([P, 512], F32, tag="psB", name="pcc12")
            prs[(1, 3)] = psB.tile([P, 512], F32, tag="psB", name="pcc13")
            for dt_i in range(DT):
                trends = []
                for b in range(BL):
                    gbt = gbts[b]
                    pad = wk1.tile([P, L + 6], F16, tag=f"d_pad{b}")
                    nc.gpsimd.memset(pad[:, 0:3], 0.0)
                    nc.gpsimd.memset(pad[:, L + 3:L + 6], 0.0)
                    nc.scalar.copy(pad[:, 3:L + 3], xin[:, b, dt_i, :])
                    sb = wk1.tile([P, L], F16, tag=f"d_s{b}")
                    tmp = wk1.tile([P, L], F16, tag=f"d_tmp{b}")
                    trend_b = wk2.tile([P, L + 2], F16, tag=f"trend{b}")
                    acc = trend_b[:, 1:L + 1]
                    nc.vector.tensor_add(sb[:], pad[:, 2:L + 2],
                                         pad[:, 3:L + 3])
                    nc.vector.tensor_add(sb[:], sb[:], pad[:, 4:L + 4])
                    nc.vector.tensor_mul(acc[:], sb[:], gbt[:, 0, :])
                    nc.vector.tensor_add(tmp[:], pad[:, 1:L + 1],
                                         pad[:, 5:L + 5])
                    nc.vector.tensor_mul(tmp[:], tmp[:], gbt[:, 1, :])
                    nc.vector.tensor_add(acc[:], acc[:], tmp[:])
                    nc.vector.tensor_add(sb[:], pad[:, 0:L], pad[:, 6:L + 6])
                    nc.vector.tensor_mul(sb[:], sb[:], gbt[:, 2, :])
                    nc.vector.tensor_add(acc[:], acc[:], sb[:])
                    nc.gpsimd.tensor_copy(trend_b[:, 0:1],
                                          trend_b[:, L:L + 1])
                    nc.gpsimd.tensor_copy(trend_b[:, L + 1:L + 2],
                                          trend_b[:, 1:2])
                    nc.vector.tensor_sub(xout_t[:, b, dt_i, :],
                                         xin[:, b, dt_i, :], acc[:])
                    trends.append(trend_b)
                for s in range(3):
                    for co in range(COT):
                        wslice = wcc3[:, s, dt_i, co * P:(co + 1) * P]
                        for b in range(BL):
                            nc.tensor.matmul(
                                prs[(b, co)][:],
                                wslice,
                                trends[b][:, s:s + L],
                                start=(s == 0 and dt_i == 0),
                                stop=(s == 2 and dt_i == DT - 1),
                                skip_group_check=True)
            for b in range(BL):
                for co in range(COT):
                    rst = wk1.tile([P, L], F16, tag=f"rtst{b}")
                    nc.vector.tensor_copy(rst[:], prs[(b, co)][:])
                    nc.sync.dma_start(rtout[widx, b, co], rst[:])
            if after_b is not None:
                for b in range(BL):
                    after_b(b)

        wq_s = wr2.tile([P, DT, D], F16, tag="wr2")
        nc.scalar.dma_start(wq_s[:], wqT[:])
        x1 = xsp.tile([P, BL, DT, L], F32, tag="xs")

        def qproj_b(b):
            x1b = wk1.tile([P, DT, L], F16, tag="xq8")
            nc.scalar.copy(x1b[:], x1[:, b])
            pqf = [psA.tile([P, P], F32, tag="psA", name=f"pqf{_i}")
                   for _i in range(DT)]
            for lc in range(LT):
                pk = psC.tile([P, 512], F32, tag="psC")
                for dc in range(DT):
                    nc.tensor.matmul(pk[:],
                                     x1b[:, dc, lc * P:(lc + 1) * P],
                                     wq_s[:, dc, :],
                                     start=(dc == 0), stop=(dc == DT - 1))
                qt = wk2.tile([P, D], F16, tag="kv_tt")
                nc.scalar.copy(qt[:], pk[:])
                for dt_i in range(DT):
                    nc.tensor.matmul(pqf[dt_i][:],
                                     qt[:, dt_i * P:(dt_i + 1) * P],
                                     fwLh_s[:, lc, :],
                                     start=(lc == 0), stop=(lc == LT - 1),
                                     skip_group_check=True)
            for dt_i in range(DT):
                nc.scalar.copy(qf_d[:, b, dt_i, :], pqf[dt_i][:])
                nc.vector.tensor_add(qf_d[:, b, dt_i, 0:1],
                                     qf_d[:, b, dt_i, 0:1],
                                     dckq_s[:, 1, dt_i, :])

        decomp(x0, x1, 0, after_b=qproj_b)

        # ============ attention =============================================
        of_sb = wk1.tile([P, BL, D], BF16, tag="om")
        for b in range(BL):
            sall = wk1.tile([DK, H, M], F32, tag="s_all")
            for hh in range(H):
                blk, half = hh // 2, (hh % 2) * DK
                pS = psB.tile([DK, M], F32, tag="psB")
                for ri in range(2):
                    nc.tensor.matmul(
                        pS[:],
                        qf_d[half:half + DK, b, blk, ri * M:(ri + 1) * M],
                        kf_d[half:half + DK, b, blk, ri * M:(ri + 1) * M],
                        start=(ri == 0), stop=(ri == 1))
                nc.vector.tensor_copy(sall[:, hh, :], pS[:])
            mx = wk1.tile([DK, H], F32, tag="s_mx")
            nc.vector.tensor_reduce(mx[:], sall[:], axis=AX.X, op=OP.max,
                                    negate=True)
            nc.vector.tensor_add(
                sall[:], sall[:],
                mx[:].unsqueeze(2).broadcast_to([DK, H, M]))
            nc.scalar.activation(sall[:], sall[:], AF.Exp)
            sm = wk1.tile([DK, H], F32, tag="s_sm")
            nc.vector.tensor_reduce(sm[:], sall[:], axis=AX.X, op=OP.add)
            rc = wk1.tile([DK, H], F32, tag="s_rc")
            nc.vector.reciprocal(rc[:], sm[:])
            nc.vector.tensor_mul(
                sall[:], sall[:],
                rc[:].unsqueeze(2).broadcast_to([DK, H, M]))
            aT = wk1.tile([DK, H, M], BF16, tag="a_T")
            for hh in range(H):
                pt = psB.tile([DK, M], F32, tag="psB")
                nc.tensor.transpose(pt[:], sall[:, hh, :], ident[0:DK, 0:DK])
                nc.vector.tensor_copy(aT[:, hh, :], pt[:])
            pof = psA.tile([P, 512], F32, tag="psA")
            for hh in range(H):
                nc.tensor.matmul(pof[0:DK, hh * DK:(hh + 1) * DK],
                                 aT[:, hh, :],
                                 vf_re[:, b, hh * DK:(hh + 1) * DK],
                                 start=True, stop=True)
                nc.tensor.matmul(pof[DK:P, hh * DK:(hh + 1) * DK],
                                 aT[:, hh, :],
                                 vf_im[:, b, hh * DK:(hh + 1) * DK],
                                 start=True, stop=True)
            nc.vector.tensor_copy(of_sb[:, b, :], pof[:])

        # idft (fm) -> wo proj + bias + residual -> x2
        wo_s = wr2.tile([P, DT, D], BF16, tag="wr2")
        nc.scalar.dma_start(wo_s[:], woT[:])
        x2 = xsp.tile([P, BL, DT, L], F32, tag="xs")
        for b in range(BL):
            apre = wk1.tile([P, DT, L], BF16, tag="ap8")
            for et in range(DT):
                pi = psA.tile([P, 512], F32, tag="psA")
                nc.tensor.matmul(pi[:], of_sb[:, b, et * P:(et + 1) * P],
                                 iv512_s[:], start=True, stop=True)
                nc.scalar.activation(apre[:, et, :], pi[:], AF.Copy,
                                     scale=262144.0)
            for et in range(DT):
                po = psA.tile([P, 512], F32, tag="psA")
                for dc in range(DT):
                    nc.tensor.matmul(po[:], wo_s[:, dc, et * P:(et + 1) * P],
                                     apre[:, dc, :],
                                     start=(dc == 0), stop=(dc == DT - 1))
                nc.vector.scalar_tensor_tensor(
                    x2[:, b, et, :], po[:], bo_s[:, et, :],
                    x1[:, b, et, :], op0=OP.add, op1=OP.add)

        # ============ decomp2 / FFN / decomp3 ===============================
        x3 = xsp.tile([P, BL, DT, L], F32, tag="xs")
        x4 = xsp.tile([P, BL, DT, L], F32, tag="xs")

        def ffn_b(b):
            x3b = wk2.tile([P, DT, L], BF16, tag="xbf")
            nc.scalar.copy(x3b[:], x3[:, b])
            y_sb = wk1.tile([P, DT, D], BF16, tag="m8k")
            for f in range(4):
                h = wk1.tile([P, FT // 4, L], BF16, tag="ffn_h")
                for fi in range(FT // 4):
                    ft = f * (FT // 4) + fi
                    ph = psC.tile([P, 512], F32, tag="psC")
                    for dc in range(DT):
                        nc.tensor.matmul(ph[:], wff1_s[:, ft, dc, :],
                                         x3b[:, dc, :],
                                         start=(dc == 0), stop=(dc == DT - 1))
                    nc.scalar.activation(h[:, fi, :], ph[:], AF.Relu)
                pys = [psA.tile([P, 512], F32, tag="psA", name=f"py{_i}")
                       for _i in range(DT)]
                for fi in range(FT // 4):
                    fc = f * (FT // 4) + fi
                    for et in range(DT):
                        nc.tensor.matmul(pys[et][:],
                                         wff2_s[:, fc, et * P:(et + 1) * P],
                                         h[:, fi, :],
                                         start=(fi == 0),
                                         stop=(fi == FT // 4 - 1))
                for et in range(DT):
                    if f == 0:
                        nc.vector.tensor_copy(y_sb[:, et, :], pys[et][:])
                    else:
                        nc.vector.tensor_add(y_sb[:, et, :], y_sb[:, et, :],
                                             pys[et][:])
            for et in range(DT):
                nc.vector.tensor_add(x4[:, b, et, :],
                                     x3[:, b, et, :],
                                     y_sb[:, et, :])

        decomp(x2, x3, 1, after_b=ffn_b)
        x5 = xsp.tile([P, BL, DT, L], F32, tag="xs")

        def out_b(b):
            nc.sync.dma_start(xout[b].rearrange("c p l -> p c l"), x5[:, b])

        decomp(x4, x5, 2, after_b=out_b)

        for cm in reversed(ctxs):
            cm.__exit__(None, None, None)

    nc.compile()
    return nc


# ---------------------------------------------------------------------------
# host side
# ---------------------------------------------------------------------------
def _fwd_basis_cols(n, modes):
    l = np.arange(n)[:, None].astype(np.float64)
    m = np.asarray(modes)[None, :].astype(np.float64)
    th = 2.0 * np.pi * l * m / n
    return np.concatenate([np.cos(th), -np.sin(th)], axis=1).astype(np.float32)


def _inv_basis(n):
    l = np.arange(n)[None, :].astype(np.float64)
    m = np.arange(M)[:, None].astype(np.float64)
    c = np.where(np.arange(M) == 0, 1.0, 2.0)[:, None]
    th = 2.0 * np.pi * l * m / n
    return np.concatenate([c * np.cos(th) / n, -c * np.sin(th) / n],
                         axis=0).astype(np.float32)


def _prep_in_maps(x, cross, feb_wr, feb_wi, wq, bq, wk, bk, wv, bv, wo, bo,
                  w_ff1, w_ff2, d1_w1, d1_b1, d1_w2, d1_b2,
                  d2_w1, d2_b1, d2_w2, d2_b2, d3_w1, d3_b1, d3_w2, d3_b2,
                  p1, p2, p3):
    bf16 = ml_dtypes.bfloat16
    f8 = ml_dtypes.float8_e4m3
    x = np.ascontiguousarray(x, np.float32)
    cross = np.ascontiguousarray(cross, np.float32)

    # token-major x: [b][p=tok][lt][d]
    xtok_full = np.ascontiguousarray(
        x.reshape(B, LT, P, D).transpose(0, 2, 1, 3)).astype(bf16)
    # feature-major x: [p=d%128][b][dc][l] (per-core slice along b)
    xfm_t = np.ascontiguousarray(x.transpose(0, 2, 1)).reshape(B, DT, P, L)
    crs_full = np.ascontiguousarray(
        cross.reshape(B, ST, P, D).transpose(0, 2, 1, 3)).astype(np.float16)

    fwL_f32 = _fwd_basis_cols(L, np.arange(M))          # [L, 2M]
    fwL_pm = np.ascontiguousarray(
        fwL_f32.reshape(LT, P, 2 * M).transpose(1, 0, 2))   # [P, LT, 2M]
    fwLb_np = fwL_pm.astype(bf16)
    fwLh_np = fwL_pm.astype(np.float16)
    fw1024r_np = np.ascontiguousarray(
        _fwd_basis_cols(S, np.arange(M)).reshape(ST, P, P).transpose(1, 0, 2)) \
        .astype(np.float16)
    iv512_np = _inv_basis(L)
    # om rows arrive as (a, n, j): row a*64 + n*8 + j holds
    # (re if a==0 else im) of global mode n*8+j
    perm = np.zeros(P, np.int64)
    for a in range(2):
        for n_i in range(NC):
            for j_i in range(MJ):
                perm[a * 64 + n_i * MJ + j_i] = a * M + n_i * MJ + j_i
    iv512_np = np.ascontiguousarray(iv512_np[perm] / 262144.0).astype(bf16)

    def pm3(w):  # [D, D] -> [P, DT, D] partition-major transposed
        return np.ascontiguousarray(
            np.asarray(w).T.reshape(DT, P, D).transpose(1, 0, 2))

    wqT_np = pm3(wq).astype(np.float16)
    wkT_np = pm3(wk).astype(np.float16)
    wvT_np = pm3(wv).astype(np.float16)
    woT_np = pm3(wo).astype(bf16)
    dcb_kq_np = np.ascontiguousarray(
        np.stack([np.asarray(bk) * S, np.asarray(bq) * L,
                  np.asarray(bv) * S]).reshape(3, DT, P, 1)
        .transpose(2, 0, 1, 3)).astype(np.float32)
    bo_np = np.ascontiguousarray(
        np.asarray(bo).reshape(DT, P, 1).transpose(1, 0, 2)).astype(np.float32)
    # [p=d][ft][dc][o]
    wff1_np = np.ascontiguousarray(
        np.asarray(w_ff1).T.reshape(DT, P, FT, P).transpose(1, 2, 0, 3)) \
        .astype(bf16)
    # [p=ff][fc][e]
    wff2_np = np.ascontiguousarray(
        np.asarray(w_ff2).T.reshape(FT, P, D).transpose(1, 0, 2)).astype(bf16)
    # [w][p=d][s][dc][co]
    wcc_np = np.zeros((3, P, 3, DT, CO), np.float16)
    for w_i, p_i in enumerate((p1, p2, p3)):
        for s in range(3):
            wcc_np[w_i, :, s] = np.ascontiguousarray(p_i[:, :, s].T) \
                .reshape(DT, P, CO).transpose(1, 0, 2)
    gw1_np = np.ascontiguousarray(
        np.stack([np.asarray(w).T.reshape(DT, P, D // 2)
                  for w in (d1_w1, d2_w1, d3_w1)])
        .transpose(2, 0, 1, 3)).astype(bf16)
    gb1_np = np.ascontiguousarray(
        np.stack([np.asarray(b_).reshape(2, P, 1)
                  for b_ in (d1_b1, d2_b1, d3_b1)])
        .transpose(2, 0, 1, 3)).astype(np.float32)
    gw2_np = np.zeros((3, 2, P, 4), np.float32)
    for i, w in enumerate((d1_w2, d2_w2, d3_w2)):
        gw2_np[i, :, :, 0:3] = np.ascontiguousarray(np.asarray(w).T) \
            .reshape(2, P, 3)
    gw2_np = np.ascontiguousarray(gw2_np.transpose(2, 0, 1, 3))
    grow_np = np.zeros((1, 16), np.float32)
    for i, b2 in enumerate((d1_b2, d2_b2, d3_b2)):
        grow_np[0, i * 4:i * 4 + 3] = np.asarray(b2, np.float32)
    grow_np[0, 12:15] = [1.0 / 3.0, 1.0 / 5.0, 1.0 / 7.0]
    # om combine: om[0:64] = t1 - t2, om[64:128] = t1 + t2
    sign_np = np.concatenate([-np.ones(64), np.ones(64)]) \
        .reshape(P, 1).astype(bf16)

    def febp(w):  # [D, D, M] -> per-core [P, MJ, DT, D] fp8
        outs = []
        for c in range(NC):
            t = np.asarray(w)[:, :, MJ * c:MJ * (c + 1)].transpose(2, 0, 1)
            t = (t * 262144.0).reshape(MJ, DT, P, D).transpose(2, 0, 1, 3)
            outs.append(np.ascontiguousarray(t).astype(f8))
        return outs

    febwr_l, febwi_l = febp(feb_wr), febp(feb_wi)

    in_maps = []
    for c in range(NC):
        bs = slice(BL * c, BL * (c + 1))
        in_maps.append(dict(
            xtok=xtok_full[bs],
            xfm=np.ascontiguousarray(xfm_t[bs].transpose(2, 0, 1, 3)),
            crs=crs_full[bs],
            fwLb=fwLb_np, fwLh=fwLh_np,
            fw1024r=fw1024r_np, iv512r=iv512_np,
            febwr=febwr_l[c], febwi=febwi_l[c],
            wqT=wqT_np, wkT=wkT_np, wvT=wvT_np, woT=woT_np,
            dcb_kq=dcb_kq_np, bo_pp=bo_np,
            wff1T=wff1_np, wff2T=wff2_np, wccT=wcc_np,
            gw1T=gw1_np, gb1=gb1_np, gw2T=gw2_np,
            grow=grow_np, sign_r=sign_np,
        ))

    return in_maps


def kernel(**inputs):
    if "nc" not in _CACHE:
        _CACHE["nc"] = _build()
    nc = _CACHE["nc"]
    in_maps = _prep_in_maps(**inputs)
    _CACHE["in_maps"] = in_maps
    res = run_bass_kernel_spmd(nc, in_maps, list(range(NC)))
    xo = np.zeros((B, L, D), np.float32)
    rt = np.zeros((B, L, CO), np.float32)
    for c in range(NC):
        r = res.results[c]
        xo[BL * c:BL * (c + 1)] = np.asarray(r["xout"]) \
            .reshape(BL, D, L).transpose(0, 2, 1)
        # rtout [3, BL, COT, P(co), L] f16 -> [BL, L, CO]
        rt[BL * c:BL * (c + 1)] = np.asarray(r["rtout"]).astype(np.float32) \
            .sum(axis=0).reshape(BL, CO, L).transpose(0, 2, 1)
    return xo, rt


# revision 31
# speedup vs baseline: 1.4138x; 1.0459x over previous
"""FEDformer-style DecoderLayer on 8 trn2 NeuronCores (Bass/Tile).

Sharding: data-parallel over batch (B=16 -> 2/core). FourierBlock mode
weights [D,D,64] are mode-sharded 8 ways. Each core DFTs its OWN 2 batches
(full 64 modes), an AllToAll redistributes spectra so core c holds all 16
batches for its 8 modes, mode matmuls run there, and a second AllToAll
returns per-mode spectra to the batch owner for the inverse DFT. All FFTs
are truncated DFT matmuls.

v2+: all DMA'd tensors pre-laid-out host-side so every DMA is contiguous
per partition (128 descriptors max); FEB/FFN/gating/circ weights
prefetched at t=0 into resident SBUF pools (FEB fp8 slots recycled for
FFN weights); om gather is 4 large DMAs with rows pre-ordered so the
re/im combine is one sign-multiply-add; circ-conv runs
weights-stationary with [co, l]-major psum accumulation across all 8
banks (both batches share each weight load); attention q/k path in fp16
(softmax logits are ~O(1500), bf16 there breaks softmax), v/o path in
bf16; gating softmax transposed via one [P,12] PE transpose + 3 small
cross-partition DMAs instead of 12 [P,1] transposes; DMA issue spread
across sync/scalar queues to avoid head-of-line blocking.
"""
import sys
sys.path.insert(0, '/opt/trn_rl_repo')
import numpy as np
import ml_dtypes

import concourse.bass as bass
import concourse.bacc as bacc
import concourse.mybir as mybir
import concourse.tile as tile
from concourse.bass_utils import run_bass_kernel_spmd
from concourse.masks import make_identity

P = 128
B, L, S, D, H, M, DFF, CO = 16, 512, 1024, 512, 8, 64, 2048, 512
NC = 8
BL = B // NC            # 2 batches/core
MJ = M // NC            # 8 modes/core
DK = D // H             # 64
DT = D // P             # 4
LT = L // P             # 4
ST = S // P             # 8
FT = DFF // P           # 16
COT = CO // P           # 4

F32 = mybir.dt.float32
F32R = mybir.dt.float32r
F16 = mybir.dt.float16
F8 = mybir.dt.float8e4
BF16 = mybir.dt.bfloat16
AF = mybir.ActivationFunctionType
OP = mybir.AluOpType
AX = mybir.AxisListType

_CACHE = {}


def _build():
    nc = bacc.Bacc("TRN2", target_bir_lowering=False, debug=False, num_devices=NC)

    def din(name, shape, dt=F32):
        return nc.dram_tensor(name, shape, dt, kind="ExternalInput")

    # all host tensors pre-laid-out partition-major -> contiguous DMAs
    xtok = din("xtok", [BL, P, LT, D], BF16)     # own x token-major
    xfm = din("xfm", [P, BL, DT, L])             # own x feature-major
    crs = din("crs", [BL, P, ST, D], F16)       # cross token-major, p-major
    fwLb = din("fwLb", [P, LT, 2 * M], BF16)     # fwd DFT basis (L)
    fwLh = din("fwLh", [P, LT, 2 * M], F16)      # same basis, f16 (Q path)
    fw1024r = din("fw1024r", [P, ST, P], F16)   # fwd DFT basis (S)
    iv512r = din("iv512r", [P, L], BF16)         # inverse DFT, A2A row order
    febwr = din("febwr", [P, MJ, DT, D], F8)
    febwi = din("febwi", [P, MJ, DT, D], F8)
    wqT = din("wqT", [P, DT, D], F16)
    wkT = din("wkT", [P, DT, D], F16)
    wvT = din("wvT", [P, DT, D], F16)
    woT = din("woT", [P, DT, D], BF16)
    dcb_kq = din("dcb_kq", [P, 3, DT, 1])        # S*bk | L*bq | S*bv cols
    bo_pp = din("bo_pp", [P, DT, 1])
    wff1T = din("wff1T", [P, FT, DT, P], BF16)   # [p=d][ft][dc][ff-col]
    wff2T = din("wff2T", [P, FT, D], BF16)       # [p=ff][fc][e]
    wccT = din("wccT", [3, P, 3, DT, CO], F16)   # [trend][p=d][shift][dc][co]
    gw1T = din("gw1T", [P, 3, DT, D // 2], BF16)
    gb1 = din("gb1", [P, 3, 2, 1])
    gw2T = din("gw2T", [P, 3, 2, 4], F32R)       # col 3 zero-pad
    grow = din("grow", [1, 16])                  # gb2 x3 (4 each) | kinv(4)
    sign_r = din("sign_r", [P, 1], BF16)

    xout = nc.dram_tensor("xout", [BL, DT, P, L], F32, kind="ExternalOutput")
    # [trend][b][co-chunk][p=co][l], f16
    rtout = nc.dram_tensor("rtout", [3, BL, COT, P, L], F16,
                           kind="ExternalOutput")

    # spectra A2A: core n sends (own 2 batches, modes of dest, re/im, all d)
    cc_sp_in = nc.dram_tensor("cc_sp_in", [NC, 2, P, BL, DT, MJ], BF16)
    cc_sp_out = nc.dram_tensor("cc_sp_out", [NC, 2, P, BL, DT, MJ], BF16)
    # om A2A: core j-owner sends per-mode products back to batch owners
    cc_om_in = nc.dram_tensor("cc_om_in", [NC, 2, BL, 2, MJ, D], BF16)
    cc_om_out = nc.dram_tensor("cc_om_out", [NC, 2, BL, 2, MJ, D], BF16)

    ctxs = []

    with tile.TileContext(nc) as tc:
        def pool(name, bufs, space="SBUF"):
            cm = tc.tile_pool(name=name, bufs=bufs, space=space)
            p = cm.__enter__()
            ctxs.append(cm)
            return p

        cp = pool("cp", 1)
        act = pool("act", 1)
        fbw = pool("fbw", 2)         # ring: FEB fp8 weights -> FFN weights
        ccw = pool("ccw", 1)         # circ-conv weights (per trend)
        xsp = pool("xsp", 2)         # rotating 2MB x-stage slots
        wr2 = pool("wr2", 2)         # rotating 1MB slots: wk,wv,wq,wo
        wk1 = pool("wk1", 1)         # single-buffered transients
        wk2 = pool("wk2", 2)         # double-buffered streams
        psA = pool("psA", 4, "PSUM")
        psC = pool("psC", 2, "PSUM")
        psB = pool("psB", 2, "PSUM")

        # ---------------- constants ----------------
        ident = cp.tile([P, P], F32, tag="ident")
        make_identity(nc, ident[:])
        warmid = psB.tile([P, P], F32, tag="psB")
        nc.tensor.transpose(warmid[:], ident[:], ident[:])

        fwLb_s = cp.tile([P, LT, 2 * M], BF16, tag="fwLb")
        nc.sync.dma_start(fwLb_s[:], fwLb[:])
        fwLh_s = cp.tile([P, LT, 2 * M], F16, tag="fwLh")
        nc.sync.dma_start(fwLh_s[:], fwLh[:])
        fw1024_s = cp.tile([P, ST, P], F16, tag="fw1024")
        nc.sync.dma_start(fw1024_s[:], fw1024r[:])
        iv512_s = cp.tile([P, L], BF16, tag="iv512")
        nc.sync.dma_start(iv512_s[:], iv512r[:])
        sign_s = cp.tile([P, 1], BF16, tag="sign")
        nc.sync.dma_start(sign_s[:], sign_r[:])
        bo_s = cp.tile([P, DT, 1], F32, tag="bo")
        nc.sync.dma_start(bo_s[:], bo_pp[:])
        gb1_s = cp.tile([P, 3, 2, 1], F32, tag="gb1")
        nc.sync.dma_start(gb1_s[:], gb1[:])
        gw2_s = cp.tile([P, 3, 2, 4], F32R, tag="gw2")
        nc.sync.dma_start(gw2_s[:], gw2T[:])
        dckq_s = cp.tile([P, 3, DT, 1], F32, tag="dckq")
        nc.sync.dma_start(dckq_s[:], dcb_kq[:])
        grow_s = cp.tile([1, 16], F32, tag="grow")
        nc.sync.dma_start(grow_s[:], grow[:])
        gbc = cp.tile([P, 16], F32, tag="gbc")
        nc.gpsimd.partition_broadcast(gbc[:], grow_s[:])
        kinv_b = gbc[:, 12:15]

        # FEB weights prefetch (contiguous 2MB each)
        fwr8 = fbw.tile([P, MJ, DT, D], F8, tag="big")
        nc.sync.dma_start(fwr8[:], febwr[:])
        fwi8 = fbw.tile([P, MJ, DT, D], F8, tag="big")
        nc.sync.dma_start(fwi8[:], febwi[:])

        # gating weights resident
        gw1_s = cp.tile([P, 3, DT, D // 2], BF16, tag="gw1s")
        nc.sync.dma_start(gw1_s[:], gw1T[:])

        # FFN weights prefetch (2MB each)
        wff1_s = fbw.tile([P, FT, DT, P], BF16, tag="big")
        nc.sync.dma_start(wff1_s[:], wff1T[:])
        wff2_s = fbw.tile([P, FT, D], BF16, tag="big")
        nc.sync.dma_start(wff2_s[:], wff2T[:])

        # ============ S1: DFT own batches (full 64 modes) ===================
        qftT = wk1.tile([P, 2, NC, BL, DT, MJ], BF16, tag="om")
        for b in range(BL):
            xb = wk2.tile([P, LT, D], BF16, tag="xtok_b")
            nc.scalar.dma_start(xb[:], xtok[b])
            for dc in range(DT):
                pd = psB.tile([P, P], F32, tag="psB")
                for lc in range(LT):
                    nc.tensor.matmul(pd[:, 0:2 * M],
                                     xb[:, lc, dc * P:(dc + 1) * P],
                                     fwLb_s[:, lc, :],
                                     start=(lc == 0), stop=(lc == LT - 1))
                nc.vector.tensor_copy(
                    qftT[:, :, :, b, dc, :],
                    pd[:, 0:2 * M].rearrange("p (r n j) -> p r n j",
                                             r=2, n=NC, j=MJ))

        # C1 cross loads first (latency-critical for PE overlap), then
        # scatter, then xfm.
        def load_cx(b):
            cxb = wk1.tile([P, ST, D], F16, tag="crs_c")
            nc.scalar.dma_start(cxb[:], crs[b])
            return cxb
        cxs = {0: load_cx(0)}
        # scatter spectra to mode owners
        for n in range(NC):
            for ri in range(2):
                nc.scalar.dma_start(cc_sp_in[n, ri], qftT[:, ri, n])
        nc.gpsimd.collective_compute(
            "AllToAll", OP.bypass, replica_groups=[list(range(NC))],
            ins=[cc_sp_in[:]], outs=[cc_sp_out[:]])

        xfm_s = xsp.tile([P, BL, DT, L], F32, tag="xs")
        nc.scalar.dma_start(xfm_s[:], xfm[:])

        # ============ C1: cross DFT (overlaps spectra A2A) ==================
        crossFd = act.tile([P, DT, BL, P], F16, tag="crossFd")
        for b in range(BL):
            cxb = cxs.pop(b) if b in cxs else load_cx(b)
            pm = psA.tile([P, 512], F32, tag="psA")
            for sc in range(ST):
                nc.tensor.matmul(pm[:], fw1024_s[:, sc, :], cxb[:, sc, :],
                                 start=(sc == 0), stop=(sc == ST - 1))
            cF = wk1.tile([P, 512], F32, tag="cF")
            nc.vector.tensor_copy(cF[:], pm[:])
            for dc in range(DT):
                pt = psB.tile([P, P], F32, tag="psB")
                nc.tensor.transpose(pt[:], cF[:, dc * P:(dc + 1) * P],
                                    ident[:])
                nc.vector.tensor_copy(crossFd[:, dc, b, :], pt[:])

        # ============ K/V proj in mode space ================================
        wk_s = wr2.tile([P, DT, D], F16, tag="wr2")
        nc.scalar.dma_start(wk_s[:], wkT[:])
        wv_s = wr2.tile([P, DT, D], F16, tag="wr2")
        nc.scalar.dma_start(wv_s[:], wvT[:])

        vf_re = act.tile([DK, BL, D], BF16, tag="vf_re")
        vf_im = act.tile([DK, BL, D], BF16, tag="vf_im")
        kf_d = act.tile([P, BL, DT, P], F16, tag="kf_d")
        qf_d = act.tile([P, BL, DT, P], F16, tag="qf_d")

        for wmat, kq, dest in ((wk_s, 0, kf_d), (wv_s, 2, None)):
            for et in range(DT):
                pk = psC.tile([P, 2 * P], F32, tag="psC")
                for dc in range(DT):
                    nc.tensor.matmul(
                        pk[:], wmat[:, dc, et * P:(et + 1) * P],
                        crossFd[:, dc].rearrange("p b m -> p (b m)"),
                        start=(dc == 0), stop=(dc == DT - 1))
                if dest is not None:
                    for b in range(BL):
                        tgt = dest[:, b, et, :]
                        nc.scalar.copy(tgt, pk[:, b * P:(b + 1) * P])
                        nc.vector.tensor_add(tgt[:, 0:1], tgt[:, 0:1],
                                             dckq_s[:, kq, et, :])
                else:
                    vtmp = wk1.tile([P, BL, P], F32, tag="vtmp")
                    nc.scalar.copy(vtmp[:], pk[:])
                    for b in range(BL):
                        nc.vector.tensor_add(vtmp[:, b, 0:1], vtmp[:, b, 0:1],
                                             dckq_s[:, kq, et, :])
                        ptr = psB.tile([DK, P], F32, tag="psB")
                        nc.tensor.transpose(ptr[:], vtmp[:, b, 0:DK],
                                            ident[:])
                        nc.vector.tensor_copy(
                            vf_re[:, b, et * P:(et + 1) * P], ptr[:])
                        pti = psB.tile([DK, P], F32, tag="psB")
                        nc.tensor.transpose(pti[:], vtmp[:, b, DK:P],
                                            ident[:])
                        nc.vector.tensor_copy(
                            vf_im[:, b, et * P:(et + 1) * P], pti[:])

        # ============ A2: per-mode matmuls on gathered spectra ==============
        qA = wk1.tile([P, 2 * BL * NC, DT, MJ], BF16, tag="om")
        for n in range(NC):
            eng = nc.sync if n % 2 == 0 else nc.scalar
            eng.dma_start(
                qA[:, n * 4:n * 4 + 4],
                cc_sp_out[n].rearrange("r p b c j -> p r b c j"))
        sgall = wk1.tile([32, 2, MJ, D], BF16, tag="stg")
        for j in range(MJ):
            g1 = psA.tile([32, 512], F32, tag="psA")
            g2 = psA.tile([32, 512], F32, tag="psA")
            for dc in range(DT):
                lh = qA[:, :, dc, j]
                nc.tensor.matmul(g1[:], lh, fwr8[:, j, dc, :],
                                 start=(dc == 0), stop=(dc == DT - 1))
                nc.tensor.matmul(g2[:], lh, fwi8[:, j, dc, :],
                                 start=(dc == 0), stop=(dc == DT - 1))
            nc.vector.tensor_copy(sgall[:, 0, j, :], g1[:])
            nc.vector.tensor_copy(sgall[:, 1, j, :], g2[:])
        nc.sync.dma_start(cc_om_in[:, :, :, 0], sgall[:, 0])
        nc.sync.dma_start(cc_om_in[:, :, :, 1], sgall[:, 1])

        nc.gpsimd.collective_compute(
            "AllToAll", OP.bypass, replica_groups=[list(range(NC))],
            ins=[cc_om_in[:]], outs=[cc_om_out[:]])

        # ============ A4: om assembly, IDFT, FEB residual ===================
        # om rows ordered (a, n, j): a=0 -> "t1-like" combination rows,
        # a=1 -> "t2-like"; host iv512 perm matches.
        x0 = xsp.tile([P, BL, DT, L], F32, tag="xs")
        for b in range(BL):
            t1 = wk1.tile([P, D], BF16, tag="a2a")
            t2 = wk1.tile([P, D], BF16, tag="a2b")
            # t1 rows: 0-63 = re*re (n,j), 64-127 = im*re (n,j)
            nc.sync.dma_start(t1[0:64], cc_om_out[:, 0, b, 0])
            nc.sync.dma_start(t1[64:128], cc_om_out[:, 1, b, 0])
            # t2 rows (half-swapped): 0-63 = im*im, 64-127 = re*im
            nc.sync.dma_start(t2[0:64], cc_om_out[:, 1, b, 1])
            nc.sync.dma_start(t2[64:128], cc_om_out[:, 0, b, 1])
            om_t = wk1.tile([P, BL, D], BF16, tag="om", name="om_t")
            om = om_t[:, 0, :]
            # om[0:64] = t1 - t2 (re part), om[64:128] = t1 + t2 (im part)
            nc.vector.scalar_tensor_tensor(om[:], t2[:], sign_s[:], t1[:],
                                           op0=OP.mult, op1=OP.add)
            for et in range(DT):
                pi = psA.tile([P, 512], F32, tag="psA")
                nc.tensor.matmul(pi[:], om[:, et * P:(et + 1) * P], iv512_s[:],
                                 start=True, stop=True)
                nc.vector.tensor_add(x0[:, b, et, :], xfm_s[:, b, et, :],
                                     pi[:])

        # ============ shared decomposition block ============================
        def decomp(xin, xout_t, widx, after_b=None, dc_dma=None):
            gb2_b = gbc[:, widx * 4:widx * 4 + 3]
            wcc3 = ccw.tile([P, 3, DT, CO], F16, tag="wcc3")
            nc.sync.dma_start(wcc3[:], wccT[widx])
            gbts = []
            for b in range(BL):
                xbf = wk2.tile([P, DT, L], BF16, tag="xbf")
                nc.scalar.copy(xbf[:], xin[:, b])
                h = wk1.tile([P, 2, L], F32R, tag=f"g_h{b}")
                for ht in range(2):
                    ph = psC.tile([P, 512], F32, tag="psC")
                    for dc in range(DT):
                        nc.tensor.matmul(ph[:],
                                         gw1_s[:, widx, dc,
                                               ht * P:(ht + 1) * P],
                                         xbf[:, dc, :],
                                         start=(dc == 0), stop=(dc == DT - 1))
                    nc.scalar.activation(h[:, ht, :], ph[:], AF.Relu,
                                         bias=gb1_s[:, widx, ht, :], scale=1.0)
                pg = psB.tile([P, LT, 4], F32, tag="psB")
                for lt_i in range(LT):
                    for hc in range(2):
                        nc.tensor.matmul(pg[:, lt_i, :],
                                         h[:, hc, lt_i * P:(lt_i + 1) * P],
                                         gw2_s[:, widx, hc, :],
                                         start=(hc == 0), stop=(hc == 1),
                                         skip_group_check=True)
                gt4 = wk1.tile([P, LT, 4], F32, tag=f"g_t{b}")
                nc.vector.tensor_add(
                    gt4[:, :, 0:3], pg[:, :, 0:3],
                    gb2_b.unsqueeze(1).broadcast_to([P, LT, 3]))
                mx4 = wk1.tile([P, LT], F32, tag=f"g_mx{b}")
                nc.vector.tensor_reduce(mx4[:], gt4[:, :, 0:3], axis=AX.X,
                                        op=OP.max, negate=True)
                nc.vector.tensor_add(
                    gt4[:, :, 0:3], gt4[:, :, 0:3],
                    mx4[:].unsqueeze(2).broadcast_to([P, LT, 3]))
                nc.scalar.activation(gt4[:, :, 0:3], gt4[:, :, 0:3], AF.Exp)
                sm4 = wk1.tile([P, LT], F32, tag=f"g_sm{b}")
                nc.vector.tensor_reduce(sm4[:], gt4[:, :, 0:3], axis=AX.X,
                                        op=OP.add)
                rc4 = wk1.tile([P, LT], F32, tag=f"g_rc{b}")
                nc.vector.reciprocal(rc4[:], sm4[:])
                nc.vector.tensor_mul(
                    gt4[:, :, 0:3], gt4[:, :, 0:3],
                    rc4[:].unsqueeze(2).broadcast_to([P, LT, 3]))
                nc.vector.tensor_mul(
                    gt4[:, :, 0:3], gt4[:, :, 0:3],
                    kinv_b.unsqueeze(1).broadcast_to([P, LT, 3]))
                nc.vector.tensor_add(gt4[:, :, 1:2], gt4[:, :, 1:2],
                                     gt4[:, :, 2:3])
                nc.vector.tensor_add(gt4[:, :, 0:1], gt4[:, :, 0:1],
                                     gt4[:, :, 1:2])
                gt4e = wk1.tile([P, 3, LT], F32, tag=f"g_te{b}")
                nc.vector.tensor_copy(
                    gt4e[:], gt4[:, :, 0:3].rearrange("p l e -> p e l"))
                pgt = psB.tile([12, P], F32, tag="psB")
                nc.tensor.transpose(
                    pgt[:], gt4e[:].rearrange("p a b -> p (a b)"), ident[:])
                g16 = wk1.tile([12, P], F16, tag=f"g16{b}")
                nc.vector.tensor_copy(g16[:], pgt[:])
                gfm = wk1.tile([1, 3, L], F16, tag=f"stg{b}")
                for e in range(3):
                    nc.scalar.dma_start(gfm[:, e, :],
                                        g16[e * 4:(e + 1) * 4, :])
                gbt = wk1.tile([P, 3, L], F16, tag=f"g_gb{b}")
                for e in range(3):
                    nc.gpsimd.partition_broadcast(gbt[:, e, :], gfm[:, e, :])
                gbts.append(gbt)

            # trend + circ conv: dc-outer, b-inner so circ-conv weights
            # (stationary) are shared across both batches.
            # psum: b0 -> psA x4 (co chunks), b1 -> psC x2 + psB x2
            prs = {}
            for co in range(COT):
                prs[(0, co)] = psA.tile([P, 512], F32, tag="psA",
                                        name=f"pcc0{co}")
            prs[(1, 0)] = psC.tile([P, 512], F32, tag="psC", name="pcc10")
            prs[(1, 1)] = psC.tile([P, 512], F32, tag="psC", name="pcc11")
            prs[(1, 2)] = psB.tile([P, 512], F32, tag="psB", name="pcc12")
            prs[(1, 3)] = psB.tile([P, 512], F32, tag="psB", name="pcc13")
            for dt_i in range(DT):
                trends = []
                for b in range(BL):
                    gbt = gbts[b]
                    pad = wk1.tile([P, L + 6], F16, tag=f"d_pad{b}")
                    nc.gpsimd.memset(pad[:, 0:3], 0.0)
                    nc.gpsimd.memset(pad[:, L + 3:L + 6], 0.0)
                    nc.scalar.copy(pad[:, 3:L + 3], xin[:, b, dt_i, :])
                    sb = wk1.tile([P, L], F16, tag=f"d_s{b}")
                    tmp = wk1.tile([P, L], F16, tag=f"d_tmp{b}")
                    trend_b = wk2.tile([P, L + 2], F16, tag=f"trend{b}")
                    acc = trend_b[:, 1:L + 1]
                    nc.vector.tensor_add(sb[:], pad[:, 2:L + 2],
                                         pad[:, 3:L + 3])
                    nc.vector.tensor_add(sb[:], sb[:], pad[:, 4:L + 4])
                    nc.vector.tensor_mul(acc[:], sb[:], gbt[:, 0, :])
                    nc.vector.tensor_add(tmp[:], pad[:, 1:L + 1],
                                         pad[:, 5:L + 5])
                    nc.vector.tensor_mul(tmp[:], tmp[:], gbt[:, 1, :])
                    nc.vector.tensor_add(acc[:], acc[:], tmp[:])
                    nc.vector.tensor_add(sb[:], pad[:, 0:L], pad[:, 6:L + 6])
                    nc.vector.tensor_mul(sb[:], sb[:], gbt[:, 2, :])
                    nc.vector.tensor_add(acc[:], acc[:], sb[:])
                    nc.gpsimd.tensor_copy(trend_b[:, 0:1],
                                          trend_b[:, L:L + 1])
                    nc.gpsimd.tensor_copy(trend_b[:, L + 1:L + 2],
                                          trend_b[:, 1:2])
                    nc.vector.tensor_sub(xout_t[:, b, dt_i, :],
                                         xin[:, b, dt_i, :], acc[:])
                    if dc_dma is not None:
                        dc_dma(b, dt_i)
                    trends.append(trend_b)
                for s in range(3):
                    for co in range(COT):
                        wslice = wcc3[:, s, dt_i, co * P:(co + 1) * P]
                        for b in range(BL):
                            nc.tensor.matmul(
                                prs[(b, co)][:],
                                wslice,
                                trends[b][:, s:s + L],
                                start=(s == 0 and dt_i == 0),
                                stop=(s == 2 and dt_i == DT - 1),
                                skip_group_check=True)
            for b in range(BL):
                for co in range(COT):
                    rst = wk1.tile([P, L], F16, tag=f"rtst{b}")
                    nc.vector.tensor_copy(rst[:], prs[(b, co)][:])
                    nc.sync.dma_start(rtout[widx, b, co], rst[:])
            if after_b is not None:
                for b in range(BL):
                    after_b(b)

        wq_s = wr2.tile([P, DT, D], F16, tag="wr2")
        nc.scalar.dma_start(wq_s[:], wqT[:])
        x1 = xsp.tile([P, BL, DT, L], F32, tag="xs")

        def qproj_b(b):
            x1b = wk1.tile([P, DT, L], F16, tag="xq8")
            nc.scalar.copy(x1b[:], x1[:, b])
            pqf = [psA.tile([P, P], F32, tag="psA", name=f"pqf{_i}")
                   for _i in range(DT)]
            for lc in range(LT):
                pk = psC.tile([P, 512], F32, tag="psC")
                for dc in range(DT):
                    nc.tensor.matmul(pk[:],
                                     x1b[:, dc, lc * P:(lc + 1) * P],
                                     wq_s[:, dc, :],
                                     start=(dc == 0), stop=(dc == DT - 1))
                qt = wk2.tile([P, D], F16, tag="kv_tt")
                nc.scalar.copy(qt[:], pk[:])
                for dt_i in range(DT):
                    nc.tensor.matmul(pqf[dt_i][:],
                                     qt[:, dt_i * P:(dt_i + 1) * P],
                                     fwLh_s[:, lc, :],
                                     start=(lc == 0), stop=(lc == LT - 1),
                                     skip_group_check=True)
            for dt_i in range(DT):
                nc.scalar.copy(qf_d[:, b, dt_i, :], pqf[dt_i][:])
                nc.vector.tensor_add(qf_d[:, b, dt_i, 0:1],
                                     qf_d[:, b, dt_i, 0:1],
                                     dckq_s[:, 1, dt_i, :])

        decomp(x0, x1, 0, after_b=qproj_b)

        # ============ attention =============================================
        of_sb = wk1.tile([P, BL, D], BF16, tag="om")
        salls = []
        for b in range(BL):
            sall = wk1.tile([DK, H, M], F32, tag=f"s_all{b}")
            for hh in range(H):
                blk, half = hh // 2, (hh % 2) * DK
                pS = psB.tile([DK, M], F32, tag="psB")
                for ri in range(2):
                    nc.tensor.matmul(
                        pS[:],
                        qf_d[half:half + DK, b, blk, ri * M:(ri + 1) * M],
                        kf_d[half:half + DK, b, blk, ri * M:(ri + 1) * M],
                        start=(ri == 0), stop=(ri == 1))
                nc.vector.tensor_copy(sall[:, hh, :], pS[:])
            salls.append(sall)
        for b in range(BL):
            sall = salls[b]
            mx = wk1.tile([DK, H], F32, tag=f"s_mx{b}")
            nc.vector.tensor_reduce(mx[:], sall[:], axis=AX.X, op=OP.max,
                                    negate=True)
            nc.vector.tensor_add(
                sall[:], sall[:],
                mx[:].unsqueeze(2).broadcast_to([DK, H, M]))
            nc.scalar.activation(sall[:], sall[:], AF.Exp)
            sm = wk1.tile([DK, H], F32, tag=f"s_sm{b}")
            nc.vector.tensor_reduce(sm[:], sall[:], axis=AX.X, op=OP.add)
            rc = wk1.tile([DK, H], F32, tag=f"s_rc{b}")
            nc.vector.reciprocal(rc[:], sm[:])
            nc.vector.tensor_mul(
                sall[:], sall[:],
                rc[:].unsqueeze(2).broadcast_to([DK, H, M]))
            aT = wk1.tile([DK, H, M], BF16, tag=f"a_T{b}")
            for hh in range(H):
                pt = psB.tile([DK, M], F32, tag="psB")
                nc.tensor.transpose(pt[:], sall[:, hh, :], ident[0:DK, 0:DK])
                nc.vector.tensor_copy(aT[:, hh, :], pt[:])
            pof = psA.tile([P, 512], F32, tag="psA")
            for hh in range(H):
                nc.tensor.matmul(pof[0:DK, hh * DK:(hh + 1) * DK],
                                 aT[:, hh, :],
                                 vf_re[:, b, hh * DK:(hh + 1) * DK],
                                 start=True, stop=True)
                nc.tensor.matmul(pof[DK:P, hh * DK:(hh + 1) * DK],
                                 aT[:, hh, :],
                                 vf_im[:, b, hh * DK:(hh + 1) * DK],
                                 start=True, stop=True)
            nc.vector.tensor_copy(of_sb[:, b, :], pof[:])

        # idft (fm) -> wo proj + bias + residual -> x2
        wo_s = wr2.tile([P, DT, D], BF16, tag="wr2")
        nc.scalar.dma_start(wo_s[:], woT[:])
        x2 = xsp.tile([P, BL, DT, L], F32, tag="xs")
        for b in range(BL):
            apre = wk1.tile([P, DT, L], BF16, tag="xq8")
            for et in range(DT):
                pi = psA.tile([P, 512], F32, tag="psA")
                nc.tensor.matmul(pi[:], of_sb[:, b, et * P:(et + 1) * P],
                                 iv512_s[:], start=True, stop=True)
                nc.scalar.activation(apre[:, et, :], pi[:], AF.Copy,
                                     scale=262144.0)
            for et in range(DT):
                po = psA.tile([P, 512], F32, tag="psA")
                for dc in range(DT):
                    nc.tensor.matmul(po[:], wo_s[:, dc, et * P:(et + 1) * P],
                                     apre[:, dc, :],
                                     start=(dc == 0), stop=(dc == DT - 1))
                nc.vector.scalar_tensor_tensor(
                    x2[:, b, et, :], po[:], bo_s[:, et, :],
                    x1[:, b, et, :], op0=OP.add, op1=OP.add)

        # ============ decomp2 / FFN / decomp3 ===============================
        x3 = xsp.tile([P, BL, DT, L], F32, tag="xs")
        x4 = xsp.tile([P, BL, DT, L], F32, tag="xs")

        def ffn_b(b):
            x3b = wk2.tile([P, DT, L], BF16, tag="xbf")
            nc.scalar.copy(x3b[:], x3[:, b])
            y_sb = wk1.tile([P, DT, D], BF16, tag="m8k")
            for f in range(4):
                h = wk1.tile([P, FT // 4, L], BF16, tag="ffn_h")
                for fi in range(FT // 4):
                    ft = f * (FT // 4) + fi
                    ph = psC.tile([P, 512], F32, tag="psC")
                    for dc in range(DT):
                        nc.tensor.matmul(ph[:], wff1_s[:, ft, dc, :],
                                         x3b[:, dc, :],
                                         start=(dc == 0), stop=(dc == DT - 1))
                    nc.scalar.activation(h[:, fi, :], ph[:], AF.Relu)
                pys = [psA.tile([P, 512], F32, tag="psA", name=f"py{_i}")
                       for _i in range(DT)]
                for fi in range(FT // 4):
                    fc = f * (FT // 4) + fi
                    for et in range(DT):
                        nc.tensor.matmul(pys[et][:],
                                         wff2_s[:, fc, et * P:(et + 1) * P],
                                         h[:, fi, :],
                                         start=(fi == 0),
                                         stop=(fi == FT // 4 - 1))
                for et in range(DT):
                    if f == 0:
                        nc.vector.tensor_copy(y_sb[:, et, :], pys[et][:])
                    else:
                        nc.vector.tensor_add(y_sb[:, et, :], y_sb[:, et, :],
                                             pys[et][:])
            for et in range(DT):
                nc.vector.tensor_add(x4[:, b, et, :],
                                     x3[:, b, et, :],
                                     y_sb[:, et, :])

        decomp(x2, x3, 1, after_b=ffn_b)
        x5 = xsp.tile([P, BL, DT, L], F32, tag="xs")

        def out_dc(b, dt_i):
            nc.sync.dma_start(xout[b, dt_i], x5[:, b, dt_i, :])

        decomp(x4, x5, 2, dc_dma=out_dc)

        for cm in reversed(ctxs):
            cm.__exit__(None, None, None)

    nc.compile()
    return nc


# ---------------------------------------------------------------------------
# host side
# ---------------------------------------------------------------------------
def _fwd_basis_cols(n, modes):
    l = np.arange(n)[:, None].astype(np.float64)
    m = np.asarray(modes)[None, :].astype(np.float64)
    th = 2.0 * np.pi * l * m / n
    return np.concatenate([np.cos(th), -np.sin(th)], axis=1).astype(np.float32)


def _inv_basis(n):
    l = np.arange(n)[None, :].astype(np.float64)
    m = np.arange(M)[:, None].astype(np.float64)
    c = np.where(np.arange(M) == 0, 1.0, 2.0)[:, None]
    th = 2.0 * np.pi * l * m / n
    return np.concatenate([c * np.cos(th) / n, -c * np.sin(th) / n],
                         axis=0).astype(np.float32)


def _prep_in_maps(x, cross, feb_wr, feb_wi, wq, bq, wk, bk, wv, bv, wo, bo,
                  w_ff1, w_ff2, d1_w1, d1_b1, d1_w2, d1_b2,
                  d2_w1, d2_b1, d2_w2, d2_b2, d3_w1, d3_b1, d3_w2, d3_b2,
                  p1, p2, p3):
    bf16 = ml_dtypes.bfloat16
    f8 = ml_dtypes.float8_e4m3
    x = np.ascontiguousarray(x, np.float32)
    cross = np.ascontiguousarray(cross, np.float32)

    # token-major x: [b][p=tok][lt][d]
    xtok_full = np.ascontiguousarray(
        x.reshape(B, LT, P, D).transpose(0, 2, 1, 3)).astype(bf16)
    # feature-major x: [p=d%128][b][dc][l] (per-core slice along b)
    xfm_t = np.ascontiguousarray(x.transpose(0, 2, 1)).reshape(B, DT, P, L)
    crs_full = np.ascontiguousarray(
        cross.reshape(B, ST, P, D).transpose(0, 2, 1, 3)).astype(np.float16)

    fwL_f32 = _fwd_basis_cols(L, np.arange(M))          # [L, 2M]
    fwL_pm = np.ascontiguousarray(
        fwL_f32.reshape(LT, P, 2 * M).transpose(1, 0, 2))   # [P, LT, 2M]
    fwLb_np = fwL_pm.astype(bf16)
    fwLh_np = fwL_pm.astype(np.float16)
    fw1024r_np = np.ascontiguousarray(
        _fwd_basis_cols(S, np.arange(M)).reshape(ST, P, P).transpose(1, 0, 2)) \
        .astype(np.float16)
    iv512_np = _inv_basis(L)
    # om rows arrive as (a, n, j): row a*64 + n*8 + j holds
    # (re if a==0 else im) of global mode n*8+j
    perm = np.zeros(P, np.int64)
    for a in range(2):
        for n_i in range(NC):
            for j_i in range(MJ):
                perm[a * 64 + n_i * MJ + j_i] = a * M + n_i * MJ + j_i
    iv512_np = np.ascontiguousarray(iv512_np[perm] / 262144.0).astype(bf16)

    def pm3(w):  # [D, D] -> [P, DT, D] partition-major transposed
        return np.ascontiguousarray(
            np.asarray(w).T.reshape(DT, P, D).transpose(1, 0, 2))

    wqT_np = pm3(wq).astype(np.float16)
    wkT_np = pm3(wk).astype(np.float16)
    wvT_np = pm3(wv).astype(np.float16)
    woT_np = pm3(wo).astype(bf16)
    dcb_kq_np = np.ascontiguousarray(
        np.stack([np.asarray(bk) * S, np.asarray(bq) * L,
                  np.asarray(bv) * S]).reshape(3, DT, P, 1)
        .transpose(2, 0, 1, 3)).astype(np.float32)
    bo_np = np.ascontiguousarray(
        np.asarray(bo).reshape(DT, P, 1).transpose(1, 0, 2)).astype(np.float32)
    # [p=d][ft][dc][o]
    wff1_np = np.ascontiguousarray(
        np.asarray(w_ff1).T.reshape(DT, P, FT, P).transpose(1, 2, 0, 3)) \
        .astype(bf16)
    # [p=ff][fc][e]
    wff2_np = np.ascontiguousarray(
        np.asarray(w_ff2).T.reshape(FT, P, D).transpose(1, 0, 2)).astype(bf16)
    # [w][p=d][s][dc][co]
    wcc_np = np.zeros((3, P, 3, DT, CO), np.float16)
    for w_i, p_i in enumerate((p1, p2, p3)):
        for s in range(3):
            wcc_np[w_i, :, s] = np.ascontiguousarray(p_i[:, :, s].T) \
                .reshape(DT, P, CO).transpose(1, 0, 2)
    gw1_np = np.ascontiguousarray(
        np.stack([np.asarray(w).T.reshape(DT, P, D // 2)
                  for w in (d1_w1, d2_w1, d3_w1)])
        .transpose(2, 0, 1, 3)).astype(bf16)
    gb1_np = np.ascontiguousarray(
        np.stack([np.asarray(b_).reshape(2, P, 1)
                  for b_ in (d1_b1, d2_b1, d3_b1)])
        .transpose(2, 0, 1, 3)).astype(np.float32)
    gw2_np = np.zeros((3, 2, P, 4), np.float32)
    for i, w in enumerate((d1_w2, d2_w2, d3_w2)):
        gw2_np[i, :, :, 0:3] = np.ascontiguousarray(np.asarray(w).T) \
            .reshape(2, P, 3)
    gw2_np = np.ascontiguousarray(gw2_np.transpose(2, 0, 1, 3))
    grow_np = np.zeros((1, 16), np.float32)
    for i, b2 in enumerate((d1_b2, d2_b2, d3_b2)):
        grow_np[0, i * 4:i * 4 + 3] = np.asarray(b2, np.float32)
    grow_np[0, 12:15] = [1.0 / 3.0, 1.0 / 5.0, 1.0 / 7.0]
    # om combine: om[0:64] = t1 - t2, om[64:128] = t1 + t2
    sign_np = np.concatenate([-np.ones(64), np.ones(64)]) \
        .reshape(P, 1).astype(bf16)

    def febp(w):  # [D, D, M] -> per-core [P, MJ, DT, D] fp8
        outs = []
        for c in range(NC):
            t = np.asarray(w)[:, :, MJ * c:MJ * (c + 1)].transpose(2, 0, 1)
            t = (t * 262144.0).reshape(MJ, DT, P, D).transpose(2, 0, 1, 3)
            outs.append(np.ascontiguousarray(t).astype(f8))
        return outs

    febwr_l, febwi_l = febp(feb_wr), febp(feb_wi)

    in_maps = []
    for c in range(NC):
        bs = slice(BL * c, BL * (c + 1))
        in_maps.append(dict(
            xtok=xtok_full[bs],
            xfm=np.ascontiguousarray(xfm_t[bs].transpose(2, 0, 1, 3)),
            crs=crs_full[bs],
            fwLb=fwLb_np, fwLh=fwLh_np,
            fw1024r=fw1024r_np, iv512r=iv512_np,
            febwr=febwr_l[c], febwi=febwi_l[c],
            wqT=wqT_np, wkT=wkT_np, wvT=wvT_np, woT=woT_np,
            dcb_kq=dcb_kq_np, bo_pp=bo_np,
            wff1T=wff1_np, wff2T=wff2_np, wccT=wcc_np,
            gw1T=gw1_np, gb1=gb1_np, gw2T=gw2_np,
            grow=grow_np, sign_r=sign_np,
        ))

    return in_maps


def kernel(**inputs):
    if "nc" not in _CACHE:
        _CACHE["nc"] = _build()
    nc = _CACHE["nc"]
    in_maps = _prep_in_maps(**inputs)
    _CACHE["in_maps"] = in_maps
    res = run_bass_kernel_spmd(nc, in_maps, list(range(NC)))
    xo = np.zeros((B, L, D), np.float32)
    rt = np.zeros((B, L, CO), np.float32)
    for c in range(NC):
        r = res.results[c]
        xo[BL * c:BL * (c + 1)] = np.asarray(r["xout"]) \
            .reshape(BL, D, L).transpose(0, 2, 1)
        # rtout [3, BL, COT, P(co), L] f16 -> [BL, L, CO]
        rt[BL * c:BL * (c + 1)] = np.asarray(r["rtout"]).astype(np.float32) \
            .sum(axis=0).reshape(BL, CO, L).transpose(0, 2, 1)
    return xo, rt
